# revision 42
# baseline (speedup 1.0000x reference)
"""Trainium2 Bass kernel for nn_MentionScore (v2).

Strategy: sort spans by start, shard 2048 consecutive sorted spans per core.
Each core touches a ~1.2k-token window of states/embeds. Layer-1 of the span
MLP is folded into per-token projections:
  h1[s] = relu(P1[start_s] + P2[end_s] + sum_t wg[s,t] P3[t] + WB[len_s])
with P1=states@W1a, P2=states@W1b, P3=embeds@W1c and WB=width_table@W1d+b1.

v2 changes vs baseline:
- P1/P2/P3 stay resident in SBUF (group windows 128-aligned); no DRAM
  round-trip for the projections.
- Span-group gathers run with the one-hot as the stationary matmul operand
  (f=512 moving), cutting LDWEIGHTS pressure ~4x; h1 is transposed back for
  layer 2 with PE transposes.
- Softmax built from the band identity exp(sa[s,l]) = exp(attns[start_s+l]):
  exp is taken once per token in the token pipeline; per group the weight
  matrix is band(d<=tau<=d+len)*exp(attns)*rinv via 3 fused DVE ops.
- Attention MLP (L1+L2) runs in fp8 e4m3 DoubleRow (2x tensor throughput);
  validated to add <1e-3 to final error.
- relu/bias epilogues on the Scalar engine; psum copies split Scalar/Vector.
"""

import sys
import types

import numpy as np
import ml_dtypes

import concourse.bass as bass
import concourse.mybir as mybir
from concourse.ap import AP
from concourse.tile import TileContext
from concourse.vector_clock import ScopedClock

BF = mybir.dt.bfloat16
F32 = mybir.dt.float32
F8 = mybir.dt.float8e4
AT = mybir.AluOpType
AF = mybir.ActivationFunctionType
DR = mybir.MatmulPerfMode.DoubleRow
bf16 = ml_dtypes.bfloat16
f8np = mybir.dt.np(F8)

N_CORES = 8
T, NSPAN, D, HID, LMAX, WD = 8192, 16384, 1024, 1024, 10, 20
C = NSPAN // N_CORES          # spans per core
G = C // 128                  # 128-span groups per core

FP8_ATTN = True


class PatchedTileContext(TileContext):
    """Workaround: walrus rejects the tail Drain when it carries >1 sem wait
    ("Too many sync wait commands"). Put each wait on its own NoOp instead."""

    def _drain_and_barrier(self, tick_clock, wait_clock):
        nc = self.nc
        drain_inst = nc.sync.drain()
        wait_clock.add_sem_waits(
            drain_inst.ins, ScopedClock({None: tick_clock.global_clock})
        )
        si = drain_inst.ins.sync_info
        if si is not None and si.on_wait is not None and len(si.on_wait) > 1:
            waits = list(si.on_wait)
            drain_inst.ins.sync_info = mybir.SyncInfo(
                on_wait=[waits[0]], on_update=list(si.on_update or [])
            )
            for w in waits[1:]:
                nop = nc.sync.nop()
                nop.ins.sync_info = mybir.SyncInfo(on_wait=[w], on_update=[])

        nc.all_engine_barrier()
        assert self.sems is not None
        popped = nc._tile_sem_poison_stack.pop()
        assert popped is self._sem_poison
        nc.clear_and_free_semaphores(list(self.sems.allocated().values()))
        nc.all_engine_barrier()


def _ceil128(x):
    return int(-(-int(x) // 128) * 128)


def _plan(span_starts, span_lengths):
    """Host-side sharding plan. Returns per-core data + static layout consts."""
    order = np.argsort(span_starts, kind="stable").astype(np.int64)
    ss = span_starts[order].reshape(N_CORES, C).astype(np.int64)
    sl = span_lengths[order].reshape(N_CORES, C).astype(np.int64)
    core_base = ss[:, 0].copy()
    sloc = ss - core_base[:, None]
    eloc = sloc + sl

    T_cap = _ceil128(int(eloc.max()) + 1)
    # 128-aligned, shared-across-cores group window bases
    mn = sloc[:, ::128].min(axis=0)                             # [G]
    mx = eloc.reshape(N_CORES, G, 128).max(axis=2).max(axis=0)  # [G]
    bases = (mn // 128) * 128
    kcs = -(-(mx - bases + 1) // 128)
    K_WIN = int(kcs.max()) * 128
    T_pad = max(T_cap, int((bases + kcs * 128).max()))
    d = sloc - np.repeat(bases, 128)[None, :]
    dl = d + sl
    assert d.min() >= 0 and (dl.reshape(N_CORES, G, 128).max(axis=2)
                             <= kcs[None, :] * 128 - 1).all(), "window overflow"

    # static pruning lists (shared program => OR over cores)
    need_s, need_e, need_b = [], [], []
    for g in range(G):
        dg = d[:, g * 128:(g + 1) * 128]
        dlg = dl[:, g * 128:(g + 1) * 128]
        ns, ne, nb = [], [], []
        for kk in range(int(kcs[g])):
            lo, hi = kk * 128, kk * 128 + 127
            if ((dg >= lo) & (dg <= hi)).any():
                ns.append(kk)
            if ((dlg >= lo) & (dlg <= hi)).any():
                ne.append(kk)
            if ((dg <= hi) & (dlg >= lo)).any():
                nb.append(kk)
        need_s.append(tuple(ns))
        need_e.append(tuple(ne))
        need_b.append(tuple(nb))

    return {
        "order": order,
        "core_base": core_base,
        "d": d.astype(np.float64),
        "dl": dl.astype(np.float64),
        "ln": sl.astype(np.float64),
        "T_cap": T_cap,
        "T_pad": int(T_pad),
        "K_WIN": int(K_WIN),
        "bases": [int(b) for b in bases],
        "kcs": [int(k) for k in kcs],
        "need_s": tuple(need_s),
        "need_e": tuple(need_e),
        "need_b": tuple(need_b),
    }


def _build(plan, b3val):
    """Build the single SPMD Bass program (static; shared by all 8 cores)."""
    T_cap = plan["T_cap"]
    K_WIN = plan["K_WIN"]
    bases = plan["bases"]
    kcs = plan["kcs"]
    need_s, need_e, need_b = plan["need_s"], plan["need_e"], plan["need_b"]
    TC = T_cap // 128
    KC = K_WIN // 128
    nc = bass.Bass()

    def par(name, shape, dt):
        return nc.declare_dram_parameter(name, list(shape), dt, isOutput=False)

    statesT_p = par("statesT", [D, T_cap], BF)
    embedsT_p = par("embedsT", [D, T_cap], BF)
    if FP8_ATTN:
        sT8_p = par("sT8", [D, T_cap], F8)
        aw1_p = par("aw1", [D, HID], F8)
        aw2_p = par("aw2", [HID, HID], F8)
    else:
        aw1_p = par("aw1", [D, HID], BF)
        aw2_p = par("aw2", [HID, HID], BF)
    aw3_p = par("aw3m", [128, 8], BF)
    ab1_p = par("ab1m", [128, 8], F32)
    ab2_p = par("ab2m", [128, 8], F32)
    w1a_p = par("w1a", [D, HID], BF)
    w1b_p = par("w1b", [D, HID], BF)
    w1c_p = par("w1c", [D, HID], BF)
    w1d_p = par("w1d", [WD, HID], BF)
    wtT_p = par("wtT", [WD, LMAX], BF)
    b1r_p = par("b1r", [1, HID], BF)
    w2_p = par("w2", [HID, HID], BF)
    b2_p = par("b2m", [128, 8], F32)
    w3_p = par("w3m", [128, 8], BF)
    dde_p = par("ddeflat", [1, 2 * C], F32)
    dmat_p = par("dmat", [128, G], F32)
    dlmat_p = par("dlmat", [128, G], F32)
    lenflat_p = par("lenflat", [1, C], F32)
    iotaK_p = par("iotaKf", [1, K_WIN], F32)
    iotaC_p = par("iotaC", [128, KC], F32)
    ident_p = par("ident", [128, 128], BF)
    scores_p = nc.declare_dram_parameter("scores", [1, C], F32, isOutput=True)

    with PatchedTileContext(nc) as tc:
        with (
            tc.tile_pool(name="pp", bufs=1) as pp,
            tc.tile_pool(name="wst", bufs=1) as wst,
            tc.tile_pool(name="gp", bufs=2) as gp,
            tc.tile_pool(name="ps", bufs=1, space="PSUM") as ps,
            tc.tile_pool(name="dp", bufs=1, space="DRAM") as dp,
        ):
            dma = nc.sync.dma_start
            sdma = dma  # scalar-queue triggers stall ACTs; keep all on sync
            nblocks = [(n0, min(512, T_cap - n0)) for n0 in range(0, T_cap, 512)]

            # ---------- startup-critical DMAs first ----------
            # sT8 on sync, aw1 on scalar: both HWDGE queue groups in parallel
            def load_s8(n0, nw, eng):
                s8 = []
                for kc in range(4):
                    t = wst.tile([128, 2, 512], F8, name=f"sT8w{kc}",
                                 tag=f"sT8w{kc}", bufs=1)
                    for j in range(2):
                        srcq = AP(tensor=sT8_p[:].tensor,
                                  offset=(kc * 256 + j * 128) * T_cap + n0,
                                  ap=[[T_cap, 128], [1, nw]])
                        eng(out=t[:, j, :nw], in_=srcq)
                    s8.append(t)
                return s8

            blk_tiles = {}
            if FP8_ATTN:
                blk_tiles["s8"] = load_s8(nblocks[0][0], nblocks[0][1], dma)
                aw1_t = []
                for kc in range(4):
                    t = pp.tile([128, 2, HID], F8, name=f"aw1_{kc}",
                                tag=f"aw1_{kc}")
                    for j in range(2):
                        for h2 in range(2):
                            srcq = AP(tensor=aw1_p[:].tensor,
                                      offset=(kc * 256 + j * 128) * HID
                                      + h2 * 512,
                                      ap=[[HID, 128], [1, 512]])
                            sdma(out=t[:, j, h2 * 512:(h2 + 1) * 512],
                                 in_=srcq)
                    aw1_t.append(t)
            else:
                aw1_t = []
                for k in range(8):
                    t = pp.tile([128, HID], BF, name=f"aw1_{k}", tag=f"aw1_{k}")
                    (dma if k % 2 else sdma)(
                        out=t[:], in_=aw1_p[k * 128:(k + 1) * 128, :])
                    aw1_t.append(t)

            # small biases needed by the first activations
            ab1_t = pp.tile([128, 8], F32, name="ab1", tag="ab1")
            dma(out=ab1_t[:], in_=ab1_p[:])
            ab2_t = pp.tile([128, 8], F32, name="ab2", tag="ab2")
            dma(out=ab2_t[:], in_=ab2_p[:])
            aw3_t = pp.tile([128, 8], BF, name="aw3", tag="aw3")
            dma(out=aw3_t[:], in_=aw3_p[:])

            # attn L2 weights (needed ~15us in)
            if FP8_ATTN:
                aw2_t = []
                for kc in range(4):
                    t = pp.tile([128, 2, HID], F8, name=f"aw2_{kc}",
                                tag=f"aw2_{kc}")
                    src = AP(tensor=aw2_p[:].tensor, offset=kc * 256 * HID,
                             ap=[[HID, 128], [128 * HID, 2], [1, HID]])
                    (dma if kc % 2 else sdma)(out=t[:], in_=src)
                    aw2_t.append(t)
            else:
                aw2_t = []
                for k in range(8):
                    t = pp.tile([128, HID], BF, name=f"aw2_{k}", tag=f"aw2_{k}")
                    (dma if k % 2 else sdma)(
                        out=t[:], in_=aw2_p[k * 128:(k + 1) * 128, :])
                    aw2_t.append(t)

            def wload(param, tag_prefix):
                tiles = []
                for k in range(8):
                    t = pp.tile([128, HID], BF, name=f"{tag_prefix}{k}",
                                tag=f"{tag_prefix}{k}")
                    (dma if k % 2 else sdma)(
                        out=t[:], in_=param[k * 128:(k + 1) * 128, :])
                    tiles.append(t)
                return tiles

            # P weights in first-use order (P loop below is pi-major)
            w1a_t = wload(w1a_p, "wWA")

            def load_se(n0, nw, which):
                tiles = []
                for k in range(8):
                    ts_ = wst.tile([128, 512], BF, name=f"{which}{k}",
                                   tag=f"{which}{k}", bufs=1)
                    p_ = statesT_p if which == "sTw" else embedsT_p
                    (dma if k % 2 else sdma)(
                        out=ts_[:, :nw],
                        in_=p_[k * 128:(k + 1) * 128, n0:n0 + nw])
                    tiles.append(ts_)
                return tiles

            blk_tiles["sTw"] = load_se(nblocks[0][0], nblocks[0][1], "sTw")
            w1b_t = wload(w1b_p, "wWB")
            blk_tiles["eTw"] = load_se(nblocks[0][0], nblocks[0][1], "eTw")
            w1c_t = wload(w1c_p, "wWC")

            def load_block(n0, nw):
                tiles = {}
                if FP8_ATTN:
                    tiles["s8"] = load_s8(n0, nw, dma)
                tiles["sTw"] = load_se(n0, nw, "sTw")
                tiles["eTw"] = load_se(n0, nw, "eTw")
                return tiles

            # ---------- constants / scalars ----------
            iotaK_t = pp.tile([128, K_WIN], F32, name="iotaK", tag="iotaK")
            dma(out=iotaK_t[:], in_=iotaK_p[:].partition_broadcast(128))
            iotaC_t = pp.tile([128, KC], F32, name="iotaC", tag="iotaC")
            dma(out=iotaC_t[:], in_=iotaC_p[:])
            ident_t = pp.tile([128, 128], BF, name="ident", tag="ident")
            dma(out=ident_t[:], in_=ident_p[:])
            ones16_t = pp.tile([1, 16], BF, name="ones16", tag="ones16")
            nc.vector.memset(ones16_t[:], 1.0)
            dmat_t = pp.tile([128, G], F32, name="dmat", tag="dmat")
            dma(out=dmat_t[:], in_=dmat_p[:])
            dlmat_t = pp.tile([128, G], F32, name="dlmat", tag="dlmat")
            dma(out=dlmat_t[:], in_=dlmat_p[:])
            b2_t = pp.tile([128, 8], F32, name="b2", tag="b2")
            dma(out=b2_t[:], in_=b2_p[:])
            w3_t = pp.tile([128, 8], BF, name="w3", tag="w3")
            dma(out=w3_t[:], in_=w3_p[:])
            b1r_t = pp.tile([1, HID], BF, name="b1r", tag="b1r")
            dma(out=b1r_t[:], in_=b1r_p[:])
            w1d_t = pp.tile([WD, HID], BF, name="w1d", tag="w1d")
            dma(out=w1d_t[:], in_=w1d_p[:])
            wtT_t = pp.tile([WD, 16], BF, name="wtT", tag="wtT")
            nc.vector.memset(wtT_t[:], 0.0)
            dma(out=wtT_t[:, :LMAX], in_=wtT_p[:])

            eat_dram = dp.tile([1, T_cap + K_WIN], BF, name="eat_dram",
                               tag="eat_dram")

            # ---------- P projection SBUF residents ----------
            Psb = []
            for pi in range(3):
                Psb.append([pp.tile([128, HID], BF, name=f"P{pi}_{jt}",
                                    tag=f"P{pi}_{jt}") for jt in range(TC)])

            eat_t = pp.tile([1, T_cap], BF, name="eat", tag="eat")

            # ---------- token pipeline ----------
            for bi, (n0, nw) in enumerate(nblocks):
                cur = blk_tiles
                sTw, eTw = cur["sTw"], cur["eTw"]
                # attn L1 (kc-outer so block-0 compute starts on the first
                # weight k-chunk instead of waiting for the full matrix)
                if FP8_ATTN:
                    h1a8 = [wst.tile([128, 2, 512], F8, name=f"h1a8{k}",
                                     tag=f"h1a8{k}", bufs=1) for k in range(4)]
                    for hq in (0, 4):
                        pts = [ps.tile([128, 512], F32, name="big", tag="big",
                                       bufs=4) for _ in range(4)]
                        for kc in range(4):
                            for hi_ in range(4):
                                hc = hq + hi_
                                nc.tensor.matmul(
                                    pts[hi_][:, :nw],
                                    aw1_t[kc][:, :, hc * 128:(hc + 1) * 128],
                                    cur["s8"][kc][:, :, :nw],
                                    start=(kc == 0), stop=(kc == 3),
                                    perf_mode=DR, skip_group_check=True)
                        for hi_ in range(4):
                            hc = hq + hi_
                            nc.scalar.activation(
                                h1a8[hc // 2][:, hc % 2, :nw], pts[hi_][:, :nw],
                                AF.Relu, bias=ab1_t[:, hc:hc + 1])
                else:
                    h1a = [wst.tile([128, 512], BF, name=f"h1a{k}",
                                    tag=f"h1a{k}", bufs=1) for k in range(8)]
                    for hc in range(8):
                        pt = ps.tile([128, 512], F32, name="big", tag="big",
                                     bufs=4)
                        for k in range(8):
                            nc.tensor.matmul(
                                pt[:, :nw],
                                aw1_t[k][:, hc * 128:(hc + 1) * 128],
                                sTw[k][:, :nw], start=(k == 0), stop=(k == 7))
                        nc.scalar.activation(
                            h1a[hc][:, :nw], pt[:, :nw], AF.Relu,
                            bias=ab1_t[:, hc:hc + 1])
                # prefetch next block inputs
                if bi + 1 < len(nblocks):
                    blk_tiles = load_block(*nblocks[bi + 1])
                # attn L2
                h2a = [wst.tile([128, 512], BF, name=f"h2a{k}", tag=f"h2a{k}",
                                bufs=1) for k in range(8)]
                if FP8_ATTN:
                    for hq in (0, 4):
                        pts = [ps.tile([128, 512], F32, name="big", tag="big",
                                       bufs=4) for _ in range(4)]
                        for kc in range(4):
                            for hi_ in range(4):
                                hc = hq + hi_
                                nc.tensor.matmul(
                                    pts[hi_][:, :nw],
                                    aw2_t[kc][:, :, hc * 128:(hc + 1) * 128],
                                    h1a8[kc][:, :, :nw],
                                    start=(kc == 0), stop=(kc == 3),
                                    perf_mode=DR, skip_group_check=True)
                        for hi_ in range(4):
                            hc = hq + hi_
                            nc.scalar.activation(
                                h2a[hc][:, :nw], pts[hi_][:, :nw], AF.Relu,
                                bias=ab2_t[:, hc:hc + 1])
                else:
                    for hc in range(8):
                        pt = ps.tile([128, 512], F32, name="big", tag="big",
                                     bufs=4)
                        for k in range(8):
                            nc.tensor.matmul(
                                pt[:, :nw],
                                aw2_t[k][:, hc * 128:(hc + 1) * 128],
                                h1a[k][:, :nw], start=(k == 0), stop=(k == 7))
                        nc.scalar.activation(
                            h2a[hc][:, :nw], pt[:, :nw], AF.Relu,
                            bias=ab2_t[:, hc:hc + 1])
                # attn w3 -> exp -> eat
                pt1 = ps.tile([1, 512], F32, name="big1", tag="big1", bufs=1)
                for k in range(8):
                    nc.tensor.matmul(pt1[:, :nw], aw3_t[:, k:k + 1],
                                     h2a[k][:, :nw],
                                     start=(k == 0), stop=(k == 7))
                nc.scalar.activation(eat_t[0:1, n0:n0 + nw], pt1[:, :nw],
                                     AF.Exp)
                dma(out=eat_dram[0:1, n0:n0 + nw], in_=eat_t[0:1, n0:n0 + nw])
                # P projections into SBUF residents (pi-major: matches the
                # w1a -> w1b -> w1c weight-arrival order at startup)
                for pi, (wt_, srcs) in enumerate(
                        ((w1a_t, sTw), (w1b_t, sTw), (w1c_t, eTw))):
                    for j in range(nw // 128):
                        js = slice(j * 128, (j + 1) * 128)
                        jt = (n0 + j * 128) // 128
                        for h0 in (0, 512):
                            pt = ps.tile([128, 512], F32, name="big",
                                         tag="big", bufs=4)
                            for k in range(8):
                                nc.tensor.matmul(
                                    pt[:], srcs[k][:, js],
                                    wt_[k][:, h0:h0 + 512],
                                    start=(k == 0), stop=(k == 7))
                            dst = Psb[pi][jt][:, h0:h0 + 512]
                            if pi == 2:
                                nc.scalar.copy(dst, pt[:])
                            else:
                                nc.vector.tensor_copy(out=dst, in_=pt[:])

            # zero-pad eat beyond T_cap (bands never reach there, but NaN-safe)
            zpad_t = pp.tile([1, K_WIN], BF, name="zpad", tag="zpad")
            nc.vector.memset(zpad_t[:], 0.0)
            dma(out=eat_dram[0:1, T_cap:], in_=zpad_t[0:1, :])

            # ---------- WB = width_table @ W1d + b1 -> [16, HID] ----------
            WB_t = pp.tile([16, HID], BF, name="WB", tag="WB")
            for h0 in range(0, HID, 512):
                pt = ps.tile([16, 512], F32, name="big16", tag="big16", bufs=1)
                nc.tensor.matmul(pt[:], wtT_t[:], w1d_t[:, h0:h0 + 512],
                                 start=True, stop=False)
                nc.tensor.matmul(pt[:], ones16_t[:], b1r_t[:, h0:h0 + 512],
                                 start=False, stop=True)
                nc.vector.tensor_copy(out=WB_t[:, h0:h0 + 512], in_=pt[:])

            # span-MLP L2 weights reuse the w1a slots
            w2_t = wload(w2_p, "wWA")

            # ---------- span groups ----------
            h1bT = None
            for g in range(G):
                W = kcs[g] * 128
                p0 = bases[g] // 128
                if g % 4 == 0:
                    h1bT = [gp.tile([128, 512], BF, name=f"h1bT{k}",
                                    tag=f"h1bT{k}", bufs=2) for k in range(8)]
                gcol = (g % 4) * 128

                dde = gp.tile([128, 256], F32, name="dde", tag="dde", bufs=2)
                dma(out=dde[:],
                    in_=dde_p[:, g * 256:(g + 1) * 256].partition_broadcast(128))
                eat_rep = gp.tile([128, K_WIN], BF, name="eat_rep",
                                  tag="eat_rep", bufs=2)
                dma(out=eat_rep[:, :W],
                    in_=eat_dram[0:1, bases[g]:bases[g] + W]
                    .partition_broadcast(128))
                len_rep = gp.tile([16, 128], F32, name="len_rep", tag="len_rep",
                                  bufs=2)
                dma(out=len_rep[:],
                    in_=lenflat_p[:, g * 128:(g + 1) * 128]
                    .partition_broadcast(16))

                # one-hot tiles [tau, s]
                ohS = {}
                for kk in need_s[g]:
                    t = gp.tile([128, 128], BF, name=f"ohS{kk}", tag=f"ohS{kk}",
                                bufs=2)
                    nc.vector.tensor_scalar(
                        out=t[:], in0=dde[:, :128],
                        scalar1=iotaC_t[:, kk:kk + 1], scalar2=None,
                        op0=AT.is_equal)
                    ohS[kk] = t
                ohE = {}
                for kk in need_e[g]:
                    t = gp.tile([128, 128], BF, name=f"ohE{kk}", tag=f"ohE{kk}",
                                bufs=2)
                    nc.vector.tensor_scalar(
                        out=t[:], in0=dde[:, 128:256],
                        scalar1=iotaC_t[:, kk:kk + 1], scalar2=None,
                        op0=AT.is_equal)
                    ohE[kk] = t
                ohlT = gp.tile([16, 128], BF, name="ohlT", tag="ohlT", bufs=2)
                nc.vector.tensor_scalar(
                    out=ohlT[:], in0=len_rep[:], scalar1=iotaC_t[:16, 0:1],
                    scalar2=None, op0=AT.is_equal)

                # wg [s, tau] = band * exp(attns) * rinv  (3 fused DVE ops)
                t2e = gp.tile([128, K_WIN], BF, name="t2e", tag="t2e", bufs=2)
                nc.vector.scalar_tensor_tensor(
                    out=t2e[:, :W], in0=iotaK_t[:, :W],
                    scalar=dlmat_t[:, g:g + 1], in1=eat_rep[:, :W],
                    op0=AT.is_le, op1=AT.mult)
                eband = gp.tile([128, K_WIN], BF, name="eband", tag="eband",
                                bufs=2)
                ssum = gp.tile([128, 1], F32, name="ssum", tag="ssum", bufs=2)
                nc.vector.scalar_tensor_tensor(
                    out=eband[:, :W], in0=iotaK_t[:, :W],
                    scalar=dmat_t[:, g:g + 1], in1=t2e[:, :W],
                    op0=AT.is_ge, op1=AT.mult, accum_out=ssum[:, 0:1])
                rinv = gp.tile([128, 1], F32, name="rinv", tag="rinv", bufs=2)
                nc.vector.reciprocal(rinv[:], ssum[:])
                wg = gp.tile([128, K_WIN], BF, name="wg", tag="wg", bufs=2)
                nc.vector.tensor_scalar(
                    out=wg[:, :W], in0=eband[:, :W], scalar1=rinv[:, 0:1],
                    scalar2=None, op0=AT.mult)

                # wgT via PE transpose
                wgT = {}
                for kk in need_b[g]:
                    trp = ps.tile([128, 128], BF, name="tr", tag="tr", bufs=2)
                    nc.tensor.transpose(
                        trp[:], wg[:, kk * 128:(kk + 1) * 128], ident_t[:])
                    t = gp.tile([128, 128], BF, name=f"wgT{kk}", tag=f"wgT{kk}",
                                bufs=2)
                    nc.vector.tensor_copy(out=t[:], in_=trp[:])
                    wgT[kk] = t

                # h1[s, hid] accumulation and relu
                steps = ([(ohS[kk], Psb[0][p0 + kk]) for kk in need_s[g]]
                         + [(ohE[kk], Psb[1][p0 + kk]) for kk in need_e[g]]
                         + [(ohlT, WB_t)]
                         + [(wgT[kk], Psb[2][p0 + kk]) for kk in need_b[g]])
                # h1 relu into TWO per-half tiles: whole-tile dependency
                # tracking otherwise makes every transpose wait on the LAST
                # relu ACT; split tiles let hc0-3 transpose immediately while
                # the second half's ACT finishes underneath them
                h1bh = [gp.tile([128, 512], BF, name=f"h1bh{i}",
                                tag=f"h1bh{i}", bufs=2) for i in range(2)]
                for hi, h0 in enumerate((0, 512)):
                    hp = ps.tile([128, 512], F32, name="big", tag="big", bufs=4)
                    for i, (lt, rt) in enumerate(steps):
                        nc.tensor.matmul(hp[:], lt[:], rt[:, h0:h0 + 512],
                                         start=(i == 0),
                                         stop=(i == len(steps) - 1))
                    nc.scalar.activation(h1bh[hi][:], hp[:], AF.Relu)
                for hc in range(8):
                    trp = ps.tile([128, 128], BF, name="tr", tag="tr", bufs=2)
                    nc.tensor.transpose(
                        trp[:], h1bh[hc // 4][:, (hc % 4) * 128:
                                              (hc % 4 + 1) * 128], ident_t[:])
                    dst = h1bT[hc][:, gcol:gcol + 128]
                    if hc % 2 == 0:
                        nc.scalar.copy(dst, trp[:])
                    else:
                        nc.vector.tensor_copy(out=dst, in_=trp[:])

                # every 4 groups: span-MLP L2+L3 on the 512-col block
                if g % 4 == 3:
                    b0 = (g // 4) * 512
                    h2b = [gp.tile([128, 512], BF, name=f"h2b{k}",
                                   tag=f"h2b{k}", bufs=1) for k in range(8)]
                    for h2c in range(8):
                        pt = ps.tile([128, 512], F32, name="big", tag="big",
                                     bufs=4)
                        for k in range(8):
                            nc.tensor.matmul(
                                pt[:], w2_t[k][:, h2c * 128:(h2c + 1) * 128],
                                h1bT[k][:], start=(k == 0), stop=(k == 7))
                        nc.scalar.activation(
                            h2b[h2c][:], pt[:], AF.Relu,
                            bias=b2_t[:, h2c:h2c + 1])
                    pt1 = ps.tile([1, 512], F32, name="big1", tag="big1",
                                  bufs=1)
                    for k in range(8):
                        nc.tensor.matmul(pt1[:], w3_t[:, k:k + 1], h2b[k][:],
                                         start=(k == 0), stop=(k == 7))
                    ob = gp.tile([1, 512], F32, name="ob", tag="ob", bufs=2)
                    nc.vector.tensor_scalar(out=ob[:], in0=pt1[:],
                                            scalar1=float(b3val), scalar2=None,
                                            op0=AT.add)
                    dma(out=scores_p[:, b0:b0 + 512], in_=ob[:])

    _split_waits(nc)
    return nc


def _split_waits(nc, max_waits=1):
    """This walrus build rejects instructions carrying >max_waits sem waits
    ("Too many sync wait commands"). Hoist excess waits onto same-engine
    NoOps placed immediately before the instruction — identical semantics
    (engine queues are in-order)."""
    ctr = [0]
    for f in nc.m.functions:
        for blk in f.blocks:
            out = []
            for ins in blk.instructions:
                si = getattr(ins, "sync_info", None)
                if si is not None and si.on_wait and len(si.on_wait) > max_waits:
                    waits = list(si.on_wait)
                    for w in waits[:-max_waits]:
                        ctr[0] += 1
                        nop = mybir.InstNoOp(
                            name=f"I-wsplit-{ctr[0]}", ins=[], outs=[],
                            sync_info=mybir.SyncInfo(on_wait=[w], on_update=[]),
                        )
                        nop.engine = ins.engine
                        out.append(nop)
                    ins.sync_info = mybir.SyncInfo(
                        on_wait=waits[-max_waits:],
                        on_update=list(si.on_update or []),
                    )
                out.append(ins)
            blk.instructions[:] = out
    return ctr[0]


_CACHE = {}
LAST_EXEC_NS = None
TRACE = False


def _install_ntff_shim():
    try:
        import antenv.axon_hooks  # noqa: F401
        return
    except ImportError:
        pass
    try:
        from trn_agent_boot.trn_boot import _ntff_profile_via_ctypes
        hook = _ntff_profile_via_ctypes("/opt/axon/libaxon_pjrt.so")
    except Exception:
        hook = None
    m1 = types.ModuleType("antenv")
    m2 = types.ModuleType("antenv.axon_hooks")
    m2.get_axon_ntff_profile_hook = lambda: hook
    m2.set_axon_ntff_profile_hook = lambda h: None
    m1.axon_hooks = m2
    sys.modules.setdefault("antenv", m1)
    sys.modules["antenv.axon_hooks"] = m2


def _prepare(inputs):
    inp = {k: np.asarray(v) for k, v in inputs.items()}
    ss = inp["span_starts"].astype(np.int64)
    sl = inp["span_lengths"].astype(np.int64)
    plan = _plan(ss, sl)
    T_cap, K_WIN = plan["T_cap"], plan["K_WIN"]
    KC = K_WIN // 128
    b3val = float(np.asarray(inp["score_b3"]).reshape(-1)[0])

    key = (T_cap, K_WIN, tuple(plan["bases"]), tuple(plan["kcs"]),
           plan["need_s"], plan["need_e"], plan["need_b"], b3val, FP8_ATTN)
    if key not in _CACHE:
        _CACHE[key] = _build(plan, b3val)
    nc = _CACHE[key]

    def bfc(x):
        return np.ascontiguousarray(np.asarray(x, dtype=np.float32)).astype(bf16)

    def f8c(x):
        return np.ascontiguousarray(np.asarray(x, dtype=np.float32)).astype(f8np)

    sw1 = inp["score_w1"].astype(np.float32)
    shared = {
        "aw3m": bfc(inp["attn_w3"].reshape(8, 128).T),
        "ab1m": np.ascontiguousarray(
            inp["attn_b1"].astype(np.float32).reshape(8, 128).T),
        "ab2m": np.ascontiguousarray(
            inp["attn_b2"].astype(np.float32).reshape(8, 128).T),
        "w1a": bfc(sw1[0:1024]),
        "w1b": bfc(sw1[1024:2048]),
        "w1c": bfc(sw1[2048:3072]),
        "w1d": bfc(sw1[3072:3092]),
        "wtT": bfc(inp["width_table"].T),
        "b1r": bfc(inp["score_b1"].reshape(1, HID)),
        "w2": bfc(inp["score_w2"]),
        "b2m": np.ascontiguousarray(
            inp["score_b2"].astype(np.float32).reshape(8, 128).T),
        "w3m": bfc(inp["score_w3"].reshape(8, 128).T),
        "iotaKf": np.arange(K_WIN, dtype=np.float32).reshape(1, -1),
        "iotaC": np.ascontiguousarray(
            (np.arange(128, dtype=np.float32)[:, None]
             + 128.0 * np.arange(KC, dtype=np.float32)[None, :])),
        "ident": np.eye(128, dtype=np.float32).astype(bf16),
    }
    if FP8_ATTN:
        shared["aw1"] = f8c(inp["attn_w1"])
        shared["aw2"] = f8c(inp["attn_w2"])
    else:
        shared["aw1"] = bfc(inp["attn_w1"])
        shared["aw2"] = bfc(inp["attn_w2"])

    states = inp["states"].astype(np.float32)
    embeds = inp["embeds"].astype(np.float32)
    in_maps = []
    for c in range(N_CORES):
        cb = int(plan["core_base"][c])
        stl = np.zeros((T_cap, D), np.float32)
        eml = np.zeros((T_cap, D), np.float32)
        hi = min(T, cb + T_cap)
        stl[: hi - cb] = states[cb:hi]
        eml[: hi - cb] = embeds[cb:hi]
        m = dict(shared)
        sT = np.ascontiguousarray(stl.T)
        m["statesT"] = sT.astype(bf16)
        m["embedsT"] = np.ascontiguousarray(eml.T).astype(bf16)
        if FP8_ATTN:
            m["sT8"] = sT.astype(f8np)
        d = plan["d"][c].astype(np.float32)
        dl = plan["dl"][c].astype(np.float32)
        ln = plan["ln"][c].astype(np.float32)
        m["dmat"] = np.ascontiguousarray(d.reshape(G, 128).T)
        m["dlmat"] = np.ascontiguousarray(dl.reshape(G, 128).T)
        dde = np.stack([d.reshape(G, 128), dl.reshape(G, 128)], axis=1)
        m["ddeflat"] = np.ascontiguousarray(dde.reshape(1, 2 * C))
        m["lenflat"] = ln.reshape(1, C)
        in_maps.append(m)

    return nc, in_maps, plan


def kernel(**inputs):
    global LAST_EXEC_NS
    from concourse.bass_utils import run_bass_kernel_spmd

    nc, in_maps, plan = _prepare(inputs)
    _install_ntff_shim()
    res = run_bass_kernel_spmd(nc, in_maps, list(range(N_CORES)), trace=TRACE)
    LAST_EXEC_NS = res.exec_time_ns

    out = np.empty(NSPAN, np.float32)
    for c in range(N_CORES):
        out[plan["order"][c * C:(c + 1) * C]] = np.asarray(
            res.results[c]["scores"]).reshape(-1)
    return out.reshape(NSPAN, 1)


# revision 43
# speedup vs baseline: 1.0182x; 1.0182x over previous
"""Trainium2 Bass kernel for nn_MentionScore (v2).

Strategy: sort spans by start, shard 2048 consecutive sorted spans per core.
Each core touches a ~1.2k-token window of states/embeds. Layer-1 of the span
MLP is folded into per-token projections:
  h1[s] = relu(P1[start_s] + P2[end_s] + sum_t wg[s,t] P3[t] + WB[len_s])
with P1=states@W1a, P2=states@W1b, P3=embeds@W1c and WB=width_table@W1d+b1.

v2 changes vs baseline:
- P1/P2/P3 stay resident in SBUF (group windows 128-aligned); no DRAM
  round-trip for the projections.
- Span-group gathers run with the one-hot as the stationary matmul operand
  (f=512 moving), cutting LDWEIGHTS pressure ~4x; h1 is transposed back for
  layer 2 with PE transposes.
- Softmax built from the band identity exp(sa[s,l]) = exp(attns[start_s+l]):
  exp is taken once per token in the token pipeline; per group the weight
  matrix is band(d<=tau<=d+len)*exp(attns)*rinv via 3 fused DVE ops.
- Attention MLP (L1+L2) runs in fp8 e4m3 DoubleRow (2x tensor throughput);
  validated to add <1e-3 to final error.
- relu/bias epilogues on the Scalar engine; psum copies split Scalar/Vector.
"""

import sys
import types

import numpy as np
import ml_dtypes

import concourse.bass as bass
import concourse.mybir as mybir
from concourse.ap import AP
from concourse.tile import TileContext
from concourse.vector_clock import ScopedClock

BF = mybir.dt.bfloat16
F32 = mybir.dt.float32
F8 = mybir.dt.float8e4
AT = mybir.AluOpType
AF = mybir.ActivationFunctionType
DR = mybir.MatmulPerfMode.DoubleRow
bf16 = ml_dtypes.bfloat16
f8np = mybir.dt.np(F8)

N_CORES = 8
T, NSPAN, D, HID, LMAX, WD = 8192, 16384, 1024, 1024, 10, 20
C = NSPAN // N_CORES          # spans per core
G = C // 128                  # 128-span groups per core

FP8_ATTN = True


class PatchedTileContext(TileContext):
    """Workaround: walrus rejects the tail Drain when it carries >1 sem wait
    ("Too many sync wait commands"). Put each wait on its own NoOp instead."""

    def _drain_and_barrier(self, tick_clock, wait_clock):
        nc = self.nc
        drain_inst = nc.sync.drain()
        wait_clock.add_sem_waits(
            drain_inst.ins, ScopedClock({None: tick_clock.global_clock})
        )
        si = drain_inst.ins.sync_info
        if si is not None and si.on_wait is not None and len(si.on_wait) > 1:
            waits = list(si.on_wait)
            drain_inst.ins.sync_info = mybir.SyncInfo(
                on_wait=[waits[0]], on_update=list(si.on_update or [])
            )
            for w in waits[1:]:
                nop = nc.sync.nop()
                nop.ins.sync_info = mybir.SyncInfo(on_wait=[w], on_update=[])

        nc.all_engine_barrier()
        assert self.sems is not None
        popped = nc._tile_sem_poison_stack.pop()
        assert popped is self._sem_poison
        nc.clear_and_free_semaphores(list(self.sems.allocated().values()))
        nc.all_engine_barrier()


def _ceil128(x):
    return int(-(-int(x) // 128) * 128)


def _plan(span_starts, span_lengths):
    """Host-side sharding plan. Returns per-core data + static layout consts."""
    order = np.argsort(span_starts, kind="stable").astype(np.int64)
    ss = span_starts[order].reshape(N_CORES, C).astype(np.int64)
    sl = span_lengths[order].reshape(N_CORES, C).astype(np.int64)
    core_base = ss[:, 0].copy()
    sloc = ss - core_base[:, None]
    eloc = sloc + sl

    T_cap = _ceil128(int(eloc.max()) + 1)
    # 128-aligned, shared-across-cores group window bases
    mn = sloc[:, ::128].min(axis=0)                             # [G]
    mx = eloc.reshape(N_CORES, G, 128).max(axis=2).max(axis=0)  # [G]
    bases = (mn // 128) * 128
    kcs = -(-(mx - bases + 1) // 128)
    K_WIN = int(kcs.max()) * 128
    T_pad = max(T_cap, int((bases + kcs * 128).max()))
    d = sloc - np.repeat(bases, 128)[None, :]
    dl = d + sl
    assert d.min() >= 0 and (dl.reshape(N_CORES, G, 128).max(axis=2)
                             <= kcs[None, :] * 128 - 1).all(), "window overflow"

    # static pruning lists (shared program => OR over cores)
    need_s, need_e, need_b = [], [], []
    for g in range(G):
        dg = d[:, g * 128:(g + 1) * 128]
        dlg = dl[:, g * 128:(g + 1) * 128]
        ns, ne, nb = [], [], []
        for kk in range(int(kcs[g])):
            lo, hi = kk * 128, kk * 128 + 127
            if ((dg >= lo) & (dg <= hi)).any():
                ns.append(kk)
            if ((dlg >= lo) & (dlg <= hi)).any():
                ne.append(kk)
            if ((dg <= hi) & (dlg >= lo)).any():
                nb.append(kk)
        need_s.append(tuple(ns))
        need_e.append(tuple(ne))
        need_b.append(tuple(nb))

    return {
        "order": order,
        "core_base": core_base,
        "d": d.astype(np.float64),
        "dl": dl.astype(np.float64),
        "ln": sl.astype(np.float64),
        "T_cap": T_cap,
        "T_pad": int(T_pad),
        "K_WIN": int(K_WIN),
        "bases": [int(b) for b in bases],
        "kcs": [int(k) for k in kcs],
        "need_s": tuple(need_s),
        "need_e": tuple(need_e),
        "need_b": tuple(need_b),
    }


def _build(plan, b3val):
    """Build the single SPMD Bass program (static; shared by all 8 cores)."""
    T_cap = plan["T_cap"]
    K_WIN = plan["K_WIN"]
    bases = plan["bases"]
    kcs = plan["kcs"]
    need_s, need_e, need_b = plan["need_s"], plan["need_e"], plan["need_b"]
    TC = T_cap // 128
    KC = K_WIN // 128
    nc = bass.Bass()

    def par(name, shape, dt):
        return nc.declare_dram_parameter(name, list(shape), dt, isOutput=False)

    statesT_p = par("statesT", [D, T_cap], BF)
    embedsT_p = par("embedsT", [D, T_cap], BF)
    if FP8_ATTN:
        sT8_p = par("sT8", [D, T_cap], F8)
        aw1_p = par("aw1", [D, HID], F8)
        aw2_p = par("aw2", [HID, HID], F8)
    else:
        aw1_p = par("aw1", [D, HID], BF)
        aw2_p = par("aw2", [HID, HID], BF)
    aw3_p = par("aw3m", [128, 8], BF)
    ab1_p = par("ab1m", [128, 8], F32)
    ab2_p = par("ab2m", [128, 8], F32)
    w1a_p = par("w1a", [D, HID], BF)
    w1b_p = par("w1b", [D, HID], BF)
    w1c_p = par("w1c", [D, HID], BF)
    w1d_p = par("w1d", [WD, HID], BF)
    wtT_p = par("wtT", [WD, LMAX], BF)
    b1r_p = par("b1r", [1, HID], BF)
    w2_p = par("w2", [HID, HID], BF)
    b2_p = par("b2m", [128, 8], F32)
    w3_p = par("w3m", [128, 8], BF)
    dde_p = par("ddeflat", [1, 2 * C], F32)
    dmat_p = par("dmat", [128, G], F32)
    dlmat_p = par("dlmat", [128, G], F32)
    lenflat_p = par("lenflat", [1, C], F32)
    iotaK_p = par("iotaKf", [1, K_WIN], F32)
    iotaC_p = par("iotaC", [128, KC], F32)
    ident_p = par("ident", [128, 128], BF)
    scores_p = nc.declare_dram_parameter("scores", [1, C], F32, isOutput=True)

    with PatchedTileContext(nc) as tc:
        with (
            tc.tile_pool(name="pp", bufs=1) as pp,
            tc.tile_pool(name="wst", bufs=1) as wst,
            tc.tile_pool(name="gp", bufs=2) as gp,
            tc.tile_pool(name="ps", bufs=1, space="PSUM") as ps,
            tc.tile_pool(name="dp", bufs=1, space="DRAM") as dp,
        ):
            dma = nc.sync.dma_start
            sdma = dma  # scalar-queue triggers stall ACTs; keep all on sync
            nblocks = [(n0, min(512, T_cap - n0)) for n0 in range(0, T_cap, 512)]

            # ---------- startup-critical DMAs first ----------
            # sT8 on sync, aw1 on scalar: both HWDGE queue groups in parallel
            def load_s8(n0, nw, eng):
                s8 = []
                for kc in range(4):
                    t = wst.tile([128, 2, 512], F8, name=f"sT8w{kc}",
                                 tag=f"sT8w{kc}", bufs=1)
                    src = AP(tensor=sT8_p[:].tensor,
                             offset=kc * 256 * T_cap + n0,
                             ap=[[T_cap, 128], [128 * T_cap, 2], [1, nw]])
                    eng(out=t[:, :, :nw], in_=src)
                    s8.append(t)
                return s8

            blk_tiles = {}
            if FP8_ATTN:
                blk_tiles["s8"] = load_s8(nblocks[0][0], nblocks[0][1], dma)
                aw1_t = []
                for kc in range(4):
                    t = pp.tile([128, 2, HID], F8, name=f"aw1_{kc}",
                                tag=f"aw1_{kc}")
                    src = AP(tensor=aw1_p[:].tensor, offset=kc * 256 * HID,
                             ap=[[HID, 128], [128 * HID, 2], [1, HID]])
                    sdma(out=t[:], in_=src)
                    aw1_t.append(t)
            else:
                aw1_t = []
                for k in range(8):
                    t = pp.tile([128, HID], BF, name=f"aw1_{k}", tag=f"aw1_{k}")
                    (dma if k % 2 else sdma)(
                        out=t[:], in_=aw1_p[k * 128:(k + 1) * 128, :])
                    aw1_t.append(t)

            # small biases needed by the first activations
            ab1_t = pp.tile([128, 8], F32, name="ab1", tag="ab1")
            dma(out=ab1_t[:], in_=ab1_p[:])
            ab2_t = pp.tile([128, 8], F32, name="ab2", tag="ab2")
            dma(out=ab2_t[:], in_=ab2_p[:])
            aw3_t = pp.tile([128, 8], BF, name="aw3", tag="aw3")
            dma(out=aw3_t[:], in_=aw3_p[:])

            # attn L2 weights (needed ~15us in)
            if FP8_ATTN:
                aw2_t = []
                for kc in range(4):
                    t = pp.tile([128, 2, HID], F8, name=f"aw2_{kc}",
                                tag=f"aw2_{kc}")
                    src = AP(tensor=aw2_p[:].tensor, offset=kc * 256 * HID,
                             ap=[[HID, 128], [128 * HID, 2], [1, HID]])
                    (dma if kc % 2 else sdma)(out=t[:], in_=src)
                    aw2_t.append(t)
            else:
                aw2_t = []
                for k in range(8):
                    t = pp.tile([128, HID], BF, name=f"aw2_{k}", tag=f"aw2_{k}")
                    (dma if k % 2 else sdma)(
                        out=t[:], in_=aw2_p[k * 128:(k + 1) * 128, :])
                    aw2_t.append(t)

            def wload(param, tag_prefix):
                tiles = []
                for k in range(8):
                    t = pp.tile([128, HID], BF, name=f"{tag_prefix}{k}",
                                tag=f"{tag_prefix}{k}")
                    (dma if k % 2 else sdma)(
                        out=t[:], in_=param[k * 128:(k + 1) * 128, :])
                    tiles.append(t)
                return tiles

            # P weights in first-use order (P loop below is pi-major)
            w1a_t = wload(w1a_p, "wWA")

            def load_se(n0, nw, which):
                tiles = []
                for k in range(8):
                    ts_ = wst.tile([128, 512], BF, name=f"{which}{k}",
                                   tag=f"{which}{k}", bufs=1)
                    p_ = statesT_p if which == "sTw" else embedsT_p
                    (dma if k % 2 else sdma)(
                        out=ts_[:, :nw],
                        in_=p_[k * 128:(k + 1) * 128, n0:n0 + nw])
                    tiles.append(ts_)
                return tiles

            blk_tiles["sTw"] = load_se(nblocks[0][0], nblocks[0][1], "sTw")
            w1b_t = wload(w1b_p, "wWB")
            blk_tiles["eTw"] = load_se(nblocks[0][0], nblocks[0][1], "eTw")
            w1c_t = wload(w1c_p, "wWC")

            def load_block(n0, nw):
                tiles = {}
                if FP8_ATTN:
                    tiles["s8"] = load_s8(n0, nw, dma)
                tiles["sTw"] = load_se(n0, nw, "sTw")
                tiles["eTw"] = load_se(n0, nw, "eTw")
                return tiles

            # ---------- constants / scalars ----------
            iotaK_t = pp.tile([128, K_WIN], F32, name="iotaK", tag="iotaK")
            dma(out=iotaK_t[:], in_=iotaK_p[:].partition_broadcast(128))
            iotaC_t = pp.tile([128, KC], F32, name="iotaC", tag="iotaC")
            dma(out=iotaC_t[:], in_=iotaC_p[:])
            ident_t = pp.tile([128, 128], BF, name="ident", tag="ident")
            dma(out=ident_t[:], in_=ident_p[:])
            ones16_t = pp.tile([1, 16], BF, name="ones16", tag="ones16")
            nc.vector.memset(ones16_t[:], 1.0)
            dmat_t = pp.tile([128, G], F32, name="dmat", tag="dmat")
            dma(out=dmat_t[:], in_=dmat_p[:])
            dlmat_t = pp.tile([128, G], F32, name="dlmat", tag="dlmat")
            dma(out=dlmat_t[:], in_=dlmat_p[:])
            b2_t = pp.tile([128, 8], F32, name="b2", tag="b2")
            dma(out=b2_t[:], in_=b2_p[:])
            w3_t = pp.tile([128, 8], BF, name="w3", tag="w3")
            dma(out=w3_t[:], in_=w3_p[:])
            b1r_t = pp.tile([1, HID], BF, name="b1r", tag="b1r")
            dma(out=b1r_t[:], in_=b1r_p[:])
            w1d_t = pp.tile([WD, HID], BF, name="w1d", tag="w1d")
            dma(out=w1d_t[:], in_=w1d_p[:])
            wtT_t = pp.tile([WD, 16], BF, name="wtT", tag="wtT")
            nc.vector.memset(wtT_t[:], 0.0)
            dma(out=wtT_t[:, :LMAX], in_=wtT_p[:])

            eat_dram = dp.tile([1, T_cap + K_WIN], BF, name="eat_dram",
                               tag="eat_dram")

            # ---------- P projection SBUF residents ----------
            Psb = []
            for pi in range(3):
                Psb.append([pp.tile([128, HID], BF, name=f"P{pi}_{jt}",
                                    tag=f"P{pi}_{jt}") for jt in range(TC)])

            eat_t = pp.tile([1, T_cap], BF, name="eat", tag="eat")

            # ---------- token pipeline ----------
            for bi, (n0, nw) in enumerate(nblocks):
                cur = blk_tiles
                sTw, eTw = cur["sTw"], cur["eTw"]
                # attn L1 (kc-outer so block-0 compute starts on the first
                # weight k-chunk instead of waiting for the full matrix)
                if FP8_ATTN:
                    h1a8 = [wst.tile([128, 2, 512], F8, name=f"h1a8{k}",
                                     tag=f"h1a8{k}", bufs=1) for k in range(4)]
                    for hq in (0, 4):
                        pts = [ps.tile([128, 512], F32, name="big", tag="big",
                                       bufs=4) for _ in range(4)]
                        for kc in range(4):
                            for hi_ in range(4):
                                hc = hq + hi_
                                nc.tensor.matmul(
                                    pts[hi_][:, :nw],
                                    aw1_t[kc][:, :, hc * 128:(hc + 1) * 128],
                                    cur["s8"][kc][:, :, :nw],
                                    start=(kc == 0), stop=(kc == 3),
                                    perf_mode=DR, skip_group_check=True)
                        for hi_ in range(4):
                            hc = hq + hi_
                            nc.scalar.activation(
                                h1a8[hc // 2][:, hc % 2, :nw], pts[hi_][:, :nw],
                                AF.Relu, bias=ab1_t[:, hc:hc + 1])
                else:
                    h1a = [wst.tile([128, 512], BF, name=f"h1a{k}",
                                    tag=f"h1a{k}", bufs=1) for k in range(8)]
                    for hc in range(8):
                        pt = ps.tile([128, 512], F32, name="big", tag="big",
                                     bufs=4)
                        for k in range(8):
                            nc.tensor.matmul(
                                pt[:, :nw],
                                aw1_t[k][:, hc * 128:(hc + 1) * 128],
                                sTw[k][:, :nw], start=(k == 0), stop=(k == 7))
                        nc.scalar.activation(
                            h1a[hc][:, :nw], pt[:, :nw], AF.Relu,
                            bias=ab1_t[:, hc:hc + 1])
                # prefetch next block inputs
                if bi + 1 < len(nblocks):
                    blk_tiles = load_block(*nblocks[bi + 1])
                # attn L2
                h2a = [wst.tile([128, 512], BF, name=f"h2a{k}", tag=f"h2a{k}",
                                bufs=1) for k in range(8)]
                if FP8_ATTN:
                    for hq in (0, 4):
                        pts = [ps.tile([128, 512], F32, name="big", tag="big",
                                       bufs=4) for _ in range(4)]
                        for kc in range(4):
                            for hi_ in range(4):
                                hc = hq + hi_
                                nc.tensor.matmul(
                                    pts[hi_][:, :nw],
                                    aw2_t[kc][:, :, hc * 128:(hc + 1) * 128],
                                    h1a8[kc][:, :, :nw],
                                    start=(kc == 0), stop=(kc == 3),
                                    perf_mode=DR, skip_group_check=True)
                        for hi_ in range(4):
                            hc = hq + hi_
                            nc.scalar.activation(
                                h2a[hc][:, :nw], pts[hi_][:, :nw], AF.Relu,
                                bias=ab2_t[:, hc:hc + 1])
                else:
                    for hc in range(8):
                        pt = ps.tile([128, 512], F32, name="big", tag="big",
                                     bufs=4)
                        for k in range(8):
                            nc.tensor.matmul(
                                pt[:, :nw],
                                aw2_t[k][:, hc * 128:(hc + 1) * 128],
                                h1a[k][:, :nw], start=(k == 0), stop=(k == 7))
                        nc.scalar.activation(
                            h2a[hc][:, :nw], pt[:, :nw], AF.Relu,
                            bias=ab2_t[:, hc:hc + 1])
                # attn w3 -> exp -> eat
                pt1 = ps.tile([1, 512], F32, name="big1", tag="big1", bufs=1)
                for k in range(8):
                    nc.tensor.matmul(pt1[:, :nw], aw3_t[:, k:k + 1],
                                     h2a[k][:, :nw],
                                     start=(k == 0), stop=(k == 7))
                nc.scalar.activation(eat_t[0:1, n0:n0 + nw], pt1[:, :nw],
                                     AF.Exp)
                dma(out=eat_dram[0:1, n0:n0 + nw], in_=eat_t[0:1, n0:n0 + nw])
                # P projections into SBUF residents (pi-major: matches the
                # w1a -> w1b -> w1c weight-arrival order at startup)
                for pi, (wt_, srcs) in enumerate(
                        ((w1a_t, sTw), (w1b_t, sTw), (w1c_t, eTw))):
                    for j in range(nw // 128):
                        js = slice(j * 128, (j + 1) * 128)
                        jt = (n0 + j * 128) // 128
                        for h0 in (0, 512):
                            pt = ps.tile([128, 512], F32, name="big",
                                         tag="big", bufs=4)
                            for k in range(8):
                                nc.tensor.matmul(
                                    pt[:], srcs[k][:, js],
                                    wt_[k][:, h0:h0 + 512],
                                    start=(k == 0), stop=(k == 7))
                            dst = Psb[pi][jt][:, h0:h0 + 512]
                            if pi == 2:
                                nc.scalar.copy(dst, pt[:])
                            else:
                                nc.vector.tensor_copy(out=dst, in_=pt[:])

            # zero-pad eat beyond T_cap (bands never reach there, but NaN-safe)
            zpad_t = pp.tile([1, K_WIN], BF, name="zpad", tag="zpad")
            nc.vector.memset(zpad_t[:], 0.0)
            dma(out=eat_dram[0:1, T_cap:], in_=zpad_t[0:1, :])

            # ---------- WB = width_table @ W1d + b1 -> [16, HID] ----------
            WB_t = pp.tile([16, HID], BF, name="WB", tag="WB")
            for h0 in range(0, HID, 512):
                pt = ps.tile([16, 512], F32, name="big16", tag="big16", bufs=1)
                nc.tensor.matmul(pt[:], wtT_t[:], w1d_t[:, h0:h0 + 512],
                                 start=True, stop=False)
                nc.tensor.matmul(pt[:], ones16_t[:], b1r_t[:, h0:h0 + 512],
                                 start=False, stop=True)
                nc.vector.tensor_copy(out=WB_t[:, h0:h0 + 512], in_=pt[:])

            # span-MLP L2 weights reuse the w1a slots
            w2_t = wload(w2_p, "wWA")

            # ---------- span groups ----------
            h1bT = None
            for g in range(G):
                W = kcs[g] * 128
                p0 = bases[g] // 128
                if g % 4 == 0:
                    h1bT = [gp.tile([128, 512], BF, name=f"h1bT{k}",
                                    tag=f"h1bT{k}", bufs=2) for k in range(8)]
                gcol = (g % 4) * 128

                dde = gp.tile([128, 256], F32, name="dde", tag="dde", bufs=2)
                dma(out=dde[:],
                    in_=dde_p[:, g * 256:(g + 1) * 256].partition_broadcast(128))
                eat_rep = gp.tile([128, K_WIN], BF, name="eat_rep",
                                  tag="eat_rep", bufs=2)
                dma(out=eat_rep[:, :W],
                    in_=eat_dram[0:1, bases[g]:bases[g] + W]
                    .partition_broadcast(128))
                len_rep = gp.tile([16, 128], F32, name="len_rep", tag="len_rep",
                                  bufs=2)
                dma(out=len_rep[:],
                    in_=lenflat_p[:, g * 128:(g + 1) * 128]
                    .partition_broadcast(16))

                # one-hot tiles [tau, s]
                ohS = {}
                for kk in need_s[g]:
                    t = gp.tile([128, 128], BF, name=f"ohS{kk}", tag=f"ohS{kk}",
                                bufs=2)
                    nc.vector.tensor_scalar(
                        out=t[:], in0=dde[:, :128],
                        scalar1=iotaC_t[:, kk:kk + 1], scalar2=None,
                        op0=AT.is_equal)
                    ohS[kk] = t
                ohE = {}
                for kk in need_e[g]:
                    t = gp.tile([128, 128], BF, name=f"ohE{kk}", tag=f"ohE{kk}",
                                bufs=2)
                    nc.vector.tensor_scalar(
                        out=t[:], in0=dde[:, 128:256],
                        scalar1=iotaC_t[:, kk:kk + 1], scalar2=None,
                        op0=AT.is_equal)
                    ohE[kk] = t
                ohlT = gp.tile([16, 128], BF, name="ohlT", tag="ohlT", bufs=2)
                nc.vector.tensor_scalar(
                    out=ohlT[:], in0=len_rep[:], scalar1=iotaC_t[:16, 0:1],
                    scalar2=None, op0=AT.is_equal)

                # wg [s, tau] = band * exp(attns) * rinv  (3 fused DVE ops)
                t2e = gp.tile([128, K_WIN], BF, name="t2e", tag="t2e", bufs=2)
                nc.vector.scalar_tensor_tensor(
                    out=t2e[:, :W], in0=iotaK_t[:, :W],
                    scalar=dlmat_t[:, g:g + 1], in1=eat_rep[:, :W],
                    op0=AT.is_le, op1=AT.mult)
                eband = gp.tile([128, K_WIN], BF, name="eband", tag="eband",
                                bufs=2)
                ssum = gp.tile([128, 1], F32, name="ssum", tag="ssum", bufs=2)
                nc.vector.scalar_tensor_tensor(
                    out=eband[:, :W], in0=iotaK_t[:, :W],
                    scalar=dmat_t[:, g:g + 1], in1=t2e[:, :W],
                    op0=AT.is_ge, op1=AT.mult, accum_out=ssum[:, 0:1])
                rinv = gp.tile([128, 1], F32, name="rinv", tag="rinv", bufs=2)
                nc.vector.reciprocal(rinv[:], ssum[:])
                wg = gp.tile([128, K_WIN], BF, name="wg", tag="wg", bufs=2)
                nc.vector.tensor_scalar(
                    out=wg[:, :W], in0=eband[:, :W], scalar1=rinv[:, 0:1],
                    scalar2=None, op0=AT.mult)

                # wgT via PE transpose
                wgT = {}
                for kk in need_b[g]:
                    trp = ps.tile([128, 128], BF, name="tr", tag="tr", bufs=2)
                    nc.tensor.transpose(
                        trp[:], wg[:, kk * 128:(kk + 1) * 128], ident_t[:])
                    t = gp.tile([128, 128], BF, name=f"wgT{kk}", tag=f"wgT{kk}",
                                bufs=2)
                    nc.vector.tensor_copy(out=t[:], in_=trp[:])
                    wgT[kk] = t

                # h1[s, hid] accumulation and relu
                steps = ([(ohS[kk], Psb[0][p0 + kk]) for kk in need_s[g]]
                         + [(ohE[kk], Psb[1][p0 + kk]) for kk in need_e[g]]
                         + [(ohlT, WB_t)]
                         + [(wgT[kk], Psb[2][p0 + kk]) for kk in need_b[g]])
                # h1 relu into TWO per-half tiles: whole-tile dependency
                # tracking otherwise makes every transpose wait on the LAST
                # relu ACT; split tiles let hc0-3 transpose immediately while
                # the second half's ACT finishes underneath them
                h1bh = [gp.tile([128, 512], BF, name=f"h1bh{i}",
                                tag=f"h1bh{i}", bufs=2) for i in range(2)]
                for hi, h0 in enumerate((0, 512)):
                    hp = ps.tile([128, 512], F32, name="big", tag="big", bufs=4)
                    for i, (lt, rt) in enumerate(steps):
                        nc.tensor.matmul(hp[:], lt[:], rt[:, h0:h0 + 512],
                                         start=(i == 0),
                                         stop=(i == len(steps) - 1))
                    nc.scalar.activation(h1bh[hi][:], hp[:], AF.Relu)
                for hc in range(8):
                    trp = ps.tile([128, 128], BF, name="tr", tag="tr", bufs=2)
                    nc.tensor.transpose(
                        trp[:], h1bh[hc // 4][:, (hc % 4) * 128:
                                              (hc % 4 + 1) * 128], ident_t[:])
                    dst = h1bT[hc][:, gcol:gcol + 128]
                    if hc % 2 == 0:
                        nc.scalar.copy(dst, trp[:])
                    else:
                        nc.vector.tensor_copy(out=dst, in_=trp[:])

                # every 4 groups: span-MLP L2+L3 on the 512-col block
                if g % 4 == 3:
                    b0 = (g // 4) * 512
                    h2b = [gp.tile([128, 512], BF, name=f"h2b{k}",
                                   tag=f"h2b{k}", bufs=1) for k in range(8)]
                    for h2c in range(8):
                        pt = ps.tile([128, 512], F32, name="big", tag="big",
                                     bufs=4)
                        for k in range(8):
                            nc.tensor.matmul(
                                pt[:], w2_t[k][:, h2c * 128:(h2c + 1) * 128],
                                h1bT[k][:], start=(k == 0), stop=(k == 7))
                        nc.scalar.activation(
                            h2b[h2c][:], pt[:], AF.Relu,
                            bias=b2_t[:, h2c:h2c + 1])
                    pt1 = ps.tile([1, 512], F32, name="big1", tag="big1",
                                  bufs=1)
                    for k in range(8):
                        nc.tensor.matmul(pt1[:], w3_t[:, k:k + 1], h2b[k][:],
                                         start=(k == 0), stop=(k == 7))
                    ob = gp.tile([1, 512], F32, name="ob", tag="ob", bufs=2)
                    nc.vector.tensor_scalar(out=ob[:], in0=pt1[:],
                                            scalar1=float(b3val), scalar2=None,
                                            op0=AT.add)
                    dma(out=scores_p[:, b0:b0 + 512], in_=ob[:])

    _split_waits(nc)
    return nc


def _split_waits(nc, max_waits=1):
    """This walrus build rejects instructions carrying >max_waits sem waits
    ("Too many sync wait commands"). Hoist excess waits onto same-engine
    NoOps placed immediately before the instruction — identical semantics
    (engine queues are in-order)."""
    ctr = [0]
    for f in nc.m.functions:
        for blk in f.blocks:
            out = []
            for ins in blk.instructions:
                si = getattr(ins, "sync_info", None)
                if si is not None and si.on_wait and len(si.on_wait) > max_waits:
                    waits = list(si.on_wait)
                    for w in waits[:-max_waits]:
                        ctr[0] += 1
                        nop = mybir.InstNoOp(
                            name=f"I-wsplit-{ctr[0]}", ins=[], outs=[],
                            sync_info=mybir.SyncInfo(on_wait=[w], on_update=[]),
                        )
                        nop.engine = ins.engine
                        out.append(nop)
                    ins.sync_info = mybir.SyncInfo(
                        on_wait=waits[-max_waits:],
                        on_update=list(si.on_update or []),
                    )
                out.append(ins)
            blk.instructions[:] = out
    return ctr[0]


_CACHE = {}
LAST_EXEC_NS = None
TRACE = False


def _install_ntff_shim():
    try:
        import antenv.axon_hooks  # noqa: F401
        return
    except ImportError:
        pass
    try:
        from trn_agent_boot.trn_boot import _ntff_profile_via_ctypes
        hook = _ntff_profile_via_ctypes("/opt/axon/libaxon_pjrt.so")
    except Exception:
        hook = None
    m1 = types.ModuleType("antenv")
    m2 = types.ModuleType("antenv.axon_hooks")
    m2.get_axon_ntff_profile_hook = lambda: hook
    m2.set_axon_ntff_profile_hook = lambda h: None
    m1.axon_hooks = m2
    sys.modules.setdefault("antenv", m1)
    sys.modules["antenv.axon_hooks"] = m2


def _prepare(inputs):
    inp = {k: np.asarray(v) for k, v in inputs.items()}
    ss = inp["span_starts"].astype(np.int64)
    sl = inp["span_lengths"].astype(np.int64)
    plan = _plan(ss, sl)
    T_cap, K_WIN = plan["T_cap"], plan["K_WIN"]
    KC = K_WIN // 128
    b3val = float(np.asarray(inp["score_b3"]).reshape(-1)[0])

    key = (T_cap, K_WIN, tuple(plan["bases"]), tuple(plan["kcs"]),
           plan["need_s"], plan["need_e"], plan["need_b"], b3val, FP8_ATTN)
    if key not in _CACHE:
        _CACHE[key] = _build(plan, b3val)
    nc = _CACHE[key]

    def bfc(x):
        return np.ascontiguousarray(np.asarray(x, dtype=np.float32)).astype(bf16)

    def f8c(x):
        return np.ascontiguousarray(np.asarray(x, dtype=np.float32)).astype(f8np)

    sw1 = inp["score_w1"].astype(np.float32)
    shared = {
        "aw3m": bfc(inp["attn_w3"].reshape(8, 128).T),
        "ab1m": np.ascontiguousarray(
            inp["attn_b1"].astype(np.float32).reshape(8, 128).T),
        "ab2m": np.ascontiguousarray(
            inp["attn_b2"].astype(np.float32).reshape(8, 128).T),
        "w1a": bfc(sw1[0:1024]),
        "w1b": bfc(sw1[1024:2048]),
        "w1c": bfc(sw1[2048:3072]),
        "w1d": bfc(sw1[3072:3092]),
        "wtT": bfc(inp["width_table"].T),
        "b1r": bfc(inp["score_b1"].reshape(1, HID)),
        "w2": bfc(inp["score_w2"]),
        "b2m": np.ascontiguousarray(
            inp["score_b2"].astype(np.float32).reshape(8, 128).T),
        "w3m": bfc(inp["score_w3"].reshape(8, 128).T),
        "iotaKf": np.arange(K_WIN, dtype=np.float32).reshape(1, -1),
        "iotaC": np.ascontiguousarray(
            (np.arange(128, dtype=np.float32)[:, None]
             + 128.0 * np.arange(KC, dtype=np.float32)[None, :])),
        "ident": np.eye(128, dtype=np.float32).astype(bf16),
    }
    if FP8_ATTN:
        shared["aw1"] = f8c(inp["attn_w1"])
        shared["aw2"] = f8c(inp["attn_w2"])
    else:
        shared["aw1"] = bfc(inp["attn_w1"])
        shared["aw2"] = bfc(inp["attn_w2"])

    states = inp["states"].astype(np.float32)
    embeds = inp["embeds"].astype(np.float32)
    in_maps = []
    for c in range(N_CORES):
        cb = int(plan["core_base"][c])
        stl = np.zeros((T_cap, D), np.float32)
        eml = np.zeros((T_cap, D), np.float32)
        hi = min(T, cb + T_cap)
        stl[: hi - cb] = states[cb:hi]
        eml[: hi - cb] = embeds[cb:hi]
        m = dict(shared)
        sT = np.ascontiguousarray(stl.T)
        m["statesT"] = sT.astype(bf16)
        m["embedsT"] = np.ascontiguousarray(eml.T).astype(bf16)
        if FP8_ATTN:
            m["sT8"] = sT.astype(f8np)
        d = plan["d"][c].astype(np.float32)
        dl = plan["dl"][c].astype(np.float32)
        ln = plan["ln"][c].astype(np.float32)
        m["dmat"] = np.ascontiguousarray(d.reshape(G, 128).T)
        m["dlmat"] = np.ascontiguousarray(dl.reshape(G, 128).T)
        dde = np.stack([d.reshape(G, 128), dl.reshape(G, 128)], axis=1)
        m["ddeflat"] = np.ascontiguousarray(dde.reshape(1, 2 * C))
        m["lenflat"] = ln.reshape(1, C)
        in_maps.append(m)

    return nc, in_maps, plan


def kernel(**inputs):
    global LAST_EXEC_NS
    from concourse.bass_utils import run_bass_kernel_spmd

    nc, in_maps, plan = _prepare(inputs)
    _install_ntff_shim()
    res = run_bass_kernel_spmd(nc, in_maps, list(range(N_CORES)), trace=TRACE)
    LAST_EXEC_NS = res.exec_time_ns

    out = np.empty(NSPAN, np.float32)
    for c in range(N_CORES):
        out[plan["order"][c * C:(c + 1) * C]] = np.asarray(
            res.results[c]["scores"]).reshape(-1)
    return out.reshape(NSPAN, 1)


# revision 44
# speedup vs baseline: 1.0694x; 1.0502x over previous
"""Trainium2 Bass kernel for nn_MentionScore (v2).

Strategy: sort spans by start, shard 2048 consecutive sorted spans per core.
Each core touches a ~1.2k-token window of states/embeds. Layer-1 of the span
MLP is folded into per-token projections:
  h1[s] = relu(P1[start_s] + P2[end_s] + sum_t wg[s,t] P3[t] + WB[len_s])
with P1=states@W1a, P2=states@W1b, P3=embeds@W1c and WB=width_table@W1d+b1.

v2 changes vs baseline:
- P1/P2/P3 stay resident in SBUF (group windows 128-aligned); no DRAM
  round-trip for the projections.
- Span-group gathers run with the one-hot as the stationary matmul operand
  (f=512 moving), cutting LDWEIGHTS pressure ~4x; h1 is transposed back for
  layer 2 with PE transposes.
- Softmax built from the band identity exp(sa[s,l]) = exp(attns[start_s+l]):
  exp is taken once per token in the token pipeline; per group the weight
  matrix is band(d<=tau<=d+len)*exp(attns)*rinv via 3 fused DVE ops.
- Attention MLP (L1+L2) runs in fp8 e4m3 DoubleRow (2x tensor throughput);
  validated to add <1e-3 to final error.
- relu/bias epilogues on the Scalar engine; psum copies split Scalar/Vector.
"""

import sys
import types

import numpy as np
import ml_dtypes

import concourse.bass as bass
import concourse.mybir as mybir
from concourse.ap import AP
from concourse.tile import TileContext
from concourse.vector_clock import ScopedClock

BF = mybir.dt.bfloat16
F32 = mybir.dt.float32
F8 = mybir.dt.float8e4
AT = mybir.AluOpType
AF = mybir.ActivationFunctionType
DR = mybir.MatmulPerfMode.DoubleRow
bf16 = ml_dtypes.bfloat16
f8np = mybir.dt.np(F8)

N_CORES = 8
T, NSPAN, D, HID, LMAX, WD = 8192, 16384, 1024, 1024, 10, 20
C = NSPAN // N_CORES          # spans per core
G = C // 128                  # 128-span groups per core

FP8_ATTN = True


class PatchedTileContext(TileContext):
    """Workaround: walrus rejects the tail Drain when it carries >1 sem wait
    ("Too many sync wait commands"). Put each wait on its own NoOp instead."""

    def _drain_and_barrier(self, tick_clock, wait_clock):
        nc = self.nc
        drain_inst = nc.sync.drain()
        wait_clock.add_sem_waits(
            drain_inst.ins, ScopedClock({None: tick_clock.global_clock})
        )
        si = drain_inst.ins.sync_info
        if si is not None and si.on_wait is not None and len(si.on_wait) > 1:
            waits = list(si.on_wait)
            drain_inst.ins.sync_info = mybir.SyncInfo(
                on_wait=[waits[0]], on_update=list(si.on_update or [])
            )
            for w in waits[1:]:
                nop = nc.sync.nop()
                nop.ins.sync_info = mybir.SyncInfo(on_wait=[w], on_update=[])

        nc.all_engine_barrier()
        assert self.sems is not None
        popped = nc._tile_sem_poison_stack.pop()
        assert popped is self._sem_poison
        nc.clear_and_free_semaphores(list(self.sems.allocated().values()))
        nc.all_engine_barrier()


def _ceil128(x):
    return int(-(-int(x) // 128) * 128)


def _plan(span_starts, span_lengths):
    """Host-side sharding plan. Returns per-core data + static layout consts."""
    order = np.argsort(span_starts, kind="stable").astype(np.int64)
    ss = span_starts[order].reshape(N_CORES, C).astype(np.int64)
    sl = span_lengths[order].reshape(N_CORES, C).astype(np.int64)
    core_base = ss[:, 0].copy()
    sloc = ss - core_base[:, None]
    eloc = sloc + sl

    T_cap = _ceil128(int(eloc.max()) + 1)
    # 128-aligned, shared-across-cores group window bases
    mn = sloc[:, ::128].min(axis=0)                             # [G]
    mx = eloc.reshape(N_CORES, G, 128).max(axis=2).max(axis=0)  # [G]
    bases = (mn // 128) * 128
    kcs = -(-(mx - bases + 1) // 128)
    K_WIN = int(kcs.max()) * 128
    T_pad = max(T_cap, int((bases + kcs * 128).max()))
    d = sloc - np.repeat(bases, 128)[None, :]
    dl = d + sl
    assert d.min() >= 0 and (dl.reshape(N_CORES, G, 128).max(axis=2)
                             <= kcs[None, :] * 128 - 1).all(), "window overflow"

    # static pruning lists (shared program => OR over cores)
    need_s, need_e, need_b = [], [], []
    for g in range(G):
        dg = d[:, g * 128:(g + 1) * 128]
        dlg = dl[:, g * 128:(g + 1) * 128]
        ns, ne, nb = [], [], []
        for kk in range(int(kcs[g])):
            lo, hi = kk * 128, kk * 128 + 127
            if ((dg >= lo) & (dg <= hi)).any():
                ns.append(kk)
            if ((dlg >= lo) & (dlg <= hi)).any():
                ne.append(kk)
            if ((dg <= hi) & (dlg >= lo)).any():
                nb.append(kk)
        need_s.append(tuple(ns))
        need_e.append(tuple(ne))
        need_b.append(tuple(nb))

    return {
        "order": order,
        "core_base": core_base,
        "d": d.astype(np.float64),
        "dl": dl.astype(np.float64),
        "ln": sl.astype(np.float64),
        "T_cap": T_cap,
        "T_pad": int(T_pad),
        "K_WIN": int(K_WIN),
        "bases": [int(b) for b in bases],
        "kcs": [int(k) for k in kcs],
        "need_s": tuple(need_s),
        "need_e": tuple(need_e),
        "need_b": tuple(need_b),
    }


def _build(plan, b3val):
    """Build the single SPMD Bass program (static; shared by all 8 cores)."""
    T_cap = plan["T_cap"]
    K_WIN = plan["K_WIN"]
    bases = plan["bases"]
    kcs = plan["kcs"]
    need_s, need_e, need_b = plan["need_s"], plan["need_e"], plan["need_b"]
    TC = T_cap // 128
    KC = K_WIN // 128
    nc = bass.Bass()

    def par(name, shape, dt):
        return nc.declare_dram_parameter(name, list(shape), dt, isOutput=False)

    statesT_p = par("statesT", [D, T_cap], BF)
    embedsT_p = par("embedsT", [D, T_cap], BF)
    if FP8_ATTN:
        sT8_p = par("sT8", [D, T_cap], F8)
        aw1_p = par("aw1", [D, HID], F8)
        aw2_p = par("aw2", [HID, HID], F8)
    else:
        aw1_p = par("aw1", [D, HID], BF)
        aw2_p = par("aw2", [HID, HID], BF)
    aw3_p = par("aw3m", [128, 8], BF)
    ab1_p = par("ab1m", [128, 8], F32)
    ab2_p = par("ab2m", [128, 8], F32)
    w1a_p = par("w1a", [D, HID], BF)
    w1b_p = par("w1b", [D, HID], BF)
    w1c_p = par("w1c", [D, HID], BF)
    w1d_p = par("w1d", [WD, HID], BF)
    wtT_p = par("wtT", [WD, LMAX], BF)
    b1r_p = par("b1r", [1, HID], BF)
    w2_p = par("w2", [HID, HID], BF)
    b2_p = par("b2m", [128, 8], F32)
    w3_p = par("w3m", [128, 8], BF)
    dde_p = par("ddeflat", [1, 2 * C], F32)
    dmat_p = par("dmat", [128, G], F32)
    dlmat_p = par("dlmat", [128, G], F32)
    lenflat_p = par("lenflat", [1, C], F32)
    iotaK_p = par("iotaKf", [1, K_WIN], F32)
    iotaC_p = par("iotaC", [128, KC], F32)
    ident_p = par("ident", [128, 128], BF)
    scores_p = nc.declare_dram_parameter("scores", [1, C], F32, isOutput=True)

    with PatchedTileContext(nc) as tc:
        with (
            tc.tile_pool(name="pp", bufs=1) as pp,
            tc.tile_pool(name="wst", bufs=1) as wst,
            tc.tile_pool(name="gp", bufs=2) as gp,
            tc.tile_pool(name="ps", bufs=1, space="PSUM") as ps,
            tc.tile_pool(name="dp", bufs=1, space="DRAM") as dp,
        ):
            dma = nc.sync.dma_start
            sdma = dma  # scalar-queue triggers stall ACTs; keep all on sync
            nblocks = [(n0, min(512, T_cap - n0)) for n0 in range(0, T_cap, 512)]

            # ---------- startup-critical DMAs first ----------
            # sT8 on sync, aw1 on scalar: both HWDGE queue groups in parallel
            def load_s8(n0, nw, eng):
                s8 = []
                for kc in range(4):
                    t = wst.tile([128, 2, 512], F8, name=f"sT8w{kc}",
                                 tag=f"sT8w{kc}", bufs=1)
                    src = AP(tensor=sT8_p[:].tensor,
                             offset=kc * 256 * T_cap + n0,
                             ap=[[T_cap, 128], [128 * T_cap, 2], [1, nw]])
                    eng(out=t[:, :, :nw], in_=src)
                    s8.append(t)
                return s8

            blk_tiles = {}
            if FP8_ATTN:
                blk_tiles["s8"] = load_s8(nblocks[0][0], nblocks[0][1], dma)
                aw1_t = []
                for kc in range(4):
                    t = pp.tile([128, 2, HID], F8, name=f"aw1_{kc}",
                                tag=f"aw1_{kc}")
                    src = AP(tensor=aw1_p[:].tensor, offset=kc * 256 * HID,
                             ap=[[HID, 128], [128 * HID, 2], [1, HID]])
                    sdma(out=t[:], in_=src)
                    aw1_t.append(t)
            else:
                aw1_t = []
                for k in range(8):
                    t = pp.tile([128, HID], BF, name=f"aw1_{k}", tag=f"aw1_{k}")
                    (dma if k % 2 else sdma)(
                        out=t[:], in_=aw1_p[k * 128:(k + 1) * 128, :])
                    aw1_t.append(t)

            # small biases needed by the first activations
            ab1_t = pp.tile([128, 8], F32, name="ab1", tag="ab1")
            dma(out=ab1_t[:], in_=ab1_p[:])
            ab2_t = pp.tile([128, 8], F32, name="ab2", tag="ab2")
            dma(out=ab2_t[:], in_=ab2_p[:])
            aw3_t = pp.tile([128, 8], BF, name="aw3", tag="aw3")
            dma(out=aw3_t[:], in_=aw3_p[:])

            # attn L2 weights (needed ~15us in)
            if FP8_ATTN:
                aw2_t = []
                for kc in range(4):
                    t = pp.tile([128, 2, HID], F8, name=f"aw2_{kc}",
                                tag=f"aw2_{kc}")
                    src = AP(tensor=aw2_p[:].tensor, offset=kc * 256 * HID,
                             ap=[[HID, 128], [128 * HID, 2], [1, HID]])
                    (dma if kc % 2 else sdma)(out=t[:], in_=src)
                    aw2_t.append(t)
            else:
                aw2_t = []
                for k in range(8):
                    t = pp.tile([128, HID], BF, name=f"aw2_{k}", tag=f"aw2_{k}")
                    (dma if k % 2 else sdma)(
                        out=t[:], in_=aw2_p[k * 128:(k + 1) * 128, :])
                    aw2_t.append(t)

            def wload(param, tag_prefix):
                tiles = []
                for k in range(8):
                    t = pp.tile([128, HID], BF, name=f"{tag_prefix}{k}",
                                tag=f"{tag_prefix}{k}")
                    (dma if k % 2 else sdma)(
                        out=t[:], in_=param[k * 128:(k + 1) * 128, :])
                    tiles.append(t)
                return tiles

            # P weights in first-use order (P loop below is pi-major)
            w1a_t = wload(w1a_p, "wWA")

            def load_se(n0, nw, which):
                tiles = []
                for k in range(8):
                    ts_ = wst.tile([128, 512], BF, name=f"{which}{k}",
                                   tag=f"{which}{k}", bufs=1)
                    p_ = statesT_p if which == "sTw" else embedsT_p
                    (dma if k % 2 else sdma)(
                        out=ts_[:, :nw],
                        in_=p_[k * 128:(k + 1) * 128, n0:n0 + nw])
                    tiles.append(ts_)
                return tiles

            blk_tiles["sTw"] = load_se(nblocks[0][0], nblocks[0][1], "sTw")
            w1b_t = wload(w1b_p, "wWB")
            blk_tiles["eTw"] = load_se(nblocks[0][0], nblocks[0][1], "eTw")
            w1c_t = wload(w1c_p, "wWC")

            def load_block(n0, nw):
                tiles = {}
                if FP8_ATTN:
                    tiles["s8"] = load_s8(n0, nw, dma)
                tiles["sTw"] = load_se(n0, nw, "sTw")
                tiles["eTw"] = load_se(n0, nw, "eTw")
                return tiles

            # ---------- constants / scalars ----------
            iotaK_t = pp.tile([128, K_WIN], F32, name="iotaK", tag="iotaK")
            dma(out=iotaK_t[:], in_=iotaK_p[:].partition_broadcast(128))
            iotaC_t = pp.tile([128, KC], F32, name="iotaC", tag="iotaC")
            dma(out=iotaC_t[:], in_=iotaC_p[:])
            ident_t = pp.tile([128, 128], BF, name="ident", tag="ident")
            dma(out=ident_t[:], in_=ident_p[:])
            ones16_t = pp.tile([1, 16], BF, name="ones16", tag="ones16")
            nc.vector.memset(ones16_t[:], 1.0)
            dmat_t = pp.tile([128, G], F32, name="dmat", tag="dmat")
            dma(out=dmat_t[:], in_=dmat_p[:])
            dlmat_t = pp.tile([128, G], F32, name="dlmat", tag="dlmat")
            dma(out=dlmat_t[:], in_=dlmat_p[:])
            b2_t = pp.tile([128, 8], F32, name="b2", tag="b2")
            dma(out=b2_t[:], in_=b2_p[:])
            w3_t = pp.tile([128, 8], BF, name="w3", tag="w3")
            dma(out=w3_t[:], in_=w3_p[:])
            b1r_t = pp.tile([1, HID], BF, name="b1r", tag="b1r")
            dma(out=b1r_t[:], in_=b1r_p[:])
            w1d_t = pp.tile([WD, HID], BF, name="w1d", tag="w1d")
            dma(out=w1d_t[:], in_=w1d_p[:])
            wtT_t = pp.tile([WD, 16], BF, name="wtT", tag="wtT")
            nc.vector.memset(wtT_t[:], 0.0)
            dma(out=wtT_t[:, :LMAX], in_=wtT_p[:])

            eat_dram = dp.tile([1, T_cap + K_WIN], BF, name="eat_dram",
                               tag="eat_dram")

            # ---------- P projection SBUF residents ----------
            Psb = []
            for pi in range(3):
                Psb.append([pp.tile([128, HID], BF, name=f"P{pi}_{jt}",
                                    tag=f"P{pi}_{jt}") for jt in range(TC)])

            eat_t = pp.tile([1, T_cap], BF, name="eat", tag="eat")

            # ---------- token pipeline ----------
            for bi, (n0, nw) in enumerate(nblocks):
                cur = blk_tiles
                sTw, eTw = cur["sTw"], cur["eTw"]
                # attn L1 (kc-outer so block-0 compute starts on the first
                # weight k-chunk instead of waiting for the full matrix)
                if FP8_ATTN:
                    h1a8 = [wst.tile([128, 2, 512], F8, name=f"h1a8{k}",
                                     tag=f"h1a8{k}", bufs=1) for k in range(4)]
                    for hq in (0, 4):
                        pts = [ps.tile([128, 512], F32, name="big", tag="big",
                                       bufs=4) for _ in range(4)]
                        for kc in range(4):
                            for hi_ in range(4):
                                hc = hq + hi_
                                nc.tensor.matmul(
                                    pts[hi_][:, :nw],
                                    aw1_t[kc][:, :, hc * 128:(hc + 1) * 128],
                                    cur["s8"][kc][:, :, :nw],
                                    start=(kc == 0), stop=(kc == 3),
                                    perf_mode=DR, skip_group_check=True)
                        for hi_ in range(4):
                            hc = hq + hi_
                            nc.scalar.activation(
                                h1a8[hc // 2][:, hc % 2, :nw], pts[hi_][:, :nw],
                                AF.Relu, bias=ab1_t[:, hc:hc + 1])
                else:
                    h1a = [wst.tile([128, 512], BF, name=f"h1a{k}",
                                    tag=f"h1a{k}", bufs=1) for k in range(8)]
                    for hc in range(8):
                        pt = ps.tile([128, 512], F32, name="big", tag="big",
                                     bufs=4)
                        for k in range(8):
                            nc.tensor.matmul(
                                pt[:, :nw],
                                aw1_t[k][:, hc * 128:(hc + 1) * 128],
                                sTw[k][:, :nw], start=(k == 0), stop=(k == 7))
                        nc.scalar.activation(
                            h1a[hc][:, :nw], pt[:, :nw], AF.Relu,
                            bias=ab1_t[:, hc:hc + 1])
                # prefetch next block inputs
                if bi + 1 < len(nblocks):
                    blk_tiles = load_block(*nblocks[bi + 1])
                # attn L2
                h2a = [wst.tile([128, 512], BF, name=f"h2a{k}", tag=f"h2a{k}",
                                bufs=1) for k in range(8)]
                if FP8_ATTN:
                    for hq in (0, 4):
                        pts = [ps.tile([128, 512], F32, name="big", tag="big",
                                       bufs=4) for _ in range(4)]
                        for kc in range(4):
                            for hi_ in range(4):
                                hc = hq + hi_
                                nc.tensor.matmul(
                                    pts[hi_][:, :nw],
                                    aw2_t[kc][:, :, hc * 128:(hc + 1) * 128],
                                    h1a8[kc][:, :, :nw],
                                    start=(kc == 0), stop=(kc == 3),
                                    perf_mode=DR, skip_group_check=True)
                        for hi_ in range(4):
                            hc = hq + hi_
                            nc.scalar.activation(
                                h2a[hc][:, :nw], pts[hi_][:, :nw], AF.Relu,
                                bias=ab2_t[:, hc:hc + 1])
                else:
                    for hc in range(8):
                        pt = ps.tile([128, 512], F32, name="big", tag="big",
                                     bufs=4)
                        for k in range(8):
                            nc.tensor.matmul(
                                pt[:, :nw],
                                aw2_t[k][:, hc * 128:(hc + 1) * 128],
                                h1a[k][:, :nw], start=(k == 0), stop=(k == 7))
                        nc.scalar.activation(
                            h2a[hc][:, :nw], pt[:, :nw], AF.Relu,
                            bias=ab2_t[:, hc:hc + 1])
                # attn w3 -> exp -> eat
                pt1 = ps.tile([1, 512], F32, name="big1", tag="big1", bufs=1)
                for k in range(8):
                    nc.tensor.matmul(pt1[:, :nw], aw3_t[:, k:k + 1],
                                     h2a[k][:, :nw],
                                     start=(k == 0), stop=(k == 7))
                nc.scalar.activation(eat_t[0:1, n0:n0 + nw], pt1[:, :nw],
                                     AF.Exp)
                dma(out=eat_dram[0:1, n0:n0 + nw], in_=eat_t[0:1, n0:n0 + nw])
                # P projections into SBUF residents (pi-major: matches the
                # w1a -> w1b -> w1c weight-arrival order at startup)
                for pi, (wt_, srcs) in enumerate(
                        ((w1a_t, sTw), (w1b_t, sTw), (w1c_t, eTw))):
                    for j in range(nw // 128):
                        js = slice(j * 128, (j + 1) * 128)
                        jt = (n0 + j * 128) // 128
                        for h0 in (0, 512):
                            pt = ps.tile([128, 512], F32, name="big",
                                         tag="big", bufs=4)
                            for k in range(8):
                                nc.tensor.matmul(
                                    pt[:], srcs[k][:, js],
                                    wt_[k][:, h0:h0 + 512],
                                    start=(k == 0), stop=(k == 7))
                            dst = Psb[pi][jt][:, h0:h0 + 512]
                            if pi == 2:
                                nc.scalar.copy(dst, pt[:])
                            else:
                                nc.vector.tensor_copy(out=dst, in_=pt[:])

            # zero-pad eat beyond T_cap (bands never reach there, but NaN-safe)
            zpad_t = pp.tile([1, K_WIN], BF, name="zpad", tag="zpad")
            nc.vector.memset(zpad_t[:], 0.0)
            dma(out=eat_dram[0:1, T_cap:], in_=zpad_t[0:1, :])

            # ---------- WB = width_table @ W1d + b1 -> [16, HID] ----------
            WB_t = pp.tile([16, HID], BF, name="WB", tag="WB")
            for h0 in range(0, HID, 512):
                pt = ps.tile([128, 512], F32, name="big", tag="big", bufs=4)
                nc.tensor.matmul(pt[:16, :], wtT_t[:], w1d_t[:, h0:h0 + 512],
                                 start=True, stop=False)
                nc.tensor.matmul(pt[:16, :], ones16_t[:], b1r_t[:, h0:h0 + 512],
                                 start=False, stop=True)
                nc.vector.tensor_copy(out=WB_t[:, h0:h0 + 512], in_=pt[:16, :])

            # span-MLP L2 weights reuse the w1a slots
            w2_t = wload(w2_p, "wWA")

            # ---------- span groups ----------
            h1bT = None
            for g in range(G):
                W = kcs[g] * 128
                p0 = bases[g] // 128
                if g % 4 == 0:
                    h1bT = [gp.tile([128, 512], BF, name=f"h1bT{k}",
                                    tag=f"h1bT{k}", bufs=2) for k in range(8)]
                gcol = (g % 4) * 128

                dde = gp.tile([128, 256], F32, name="dde", tag="dde", bufs=2)
                dma(out=dde[:],
                    in_=dde_p[:, g * 256:(g + 1) * 256].partition_broadcast(128))
                eat_rep = gp.tile([128, K_WIN], BF, name="eat_rep",
                                  tag="eat_rep", bufs=2)
                dma(out=eat_rep[:, :W],
                    in_=eat_dram[0:1, bases[g]:bases[g] + W]
                    .partition_broadcast(128))
                len_rep = gp.tile([16, 128], F32, name="len_rep", tag="len_rep",
                                  bufs=2)
                dma(out=len_rep[:],
                    in_=lenflat_p[:, g * 128:(g + 1) * 128]
                    .partition_broadcast(16))

                # one-hot tiles [tau, s]
                ohS = {}
                for kk in need_s[g]:
                    t = gp.tile([128, 128], BF, name=f"ohS{kk}", tag=f"ohS{kk}",
                                bufs=2)
                    nc.vector.tensor_scalar(
                        out=t[:], in0=dde[:, :128],
                        scalar1=iotaC_t[:, kk:kk + 1], scalar2=None,
                        op0=AT.is_equal)
                    ohS[kk] = t
                ohE = {}
                for kk in need_e[g]:
                    t = gp.tile([128, 128], BF, name=f"ohE{kk}", tag=f"ohE{kk}",
                                bufs=2)
                    nc.vector.tensor_scalar(
                        out=t[:], in0=dde[:, 128:256],
                        scalar1=iotaC_t[:, kk:kk + 1], scalar2=None,
                        op0=AT.is_equal)
                    ohE[kk] = t
                ohlT = gp.tile([16, 128], BF, name="ohlT", tag="ohlT", bufs=2)
                nc.vector.tensor_scalar(
                    out=ohlT[:], in0=len_rep[:], scalar1=iotaC_t[:16, 0:1],
                    scalar2=None, op0=AT.is_equal)

                # wg [s, tau] = band * exp(attns) * rinv  (3 fused DVE ops)
                t2e = gp.tile([128, K_WIN], BF, name="t2e", tag="t2e", bufs=2)
                nc.vector.scalar_tensor_tensor(
                    out=t2e[:, :W], in0=iotaK_t[:, :W],
                    scalar=dlmat_t[:, g:g + 1], in1=eat_rep[:, :W],
                    op0=AT.is_le, op1=AT.mult)
                eband = gp.tile([128, K_WIN], BF, name="eband", tag="eband",
                                bufs=2)
                ssum = gp.tile([128, 1], F32, name="ssum", tag="ssum", bufs=2)
                nc.vector.scalar_tensor_tensor(
                    out=eband[:, :W], in0=iotaK_t[:, :W],
                    scalar=dmat_t[:, g:g + 1], in1=t2e[:, :W],
                    op0=AT.is_ge, op1=AT.mult, accum_out=ssum[:, 0:1])
                rinv = gp.tile([128, 1], F32, name="rinv", tag="rinv", bufs=2)
                nc.vector.reciprocal(rinv[:], ssum[:])
                wg = gp.tile([128, K_WIN], BF, name="wg", tag="wg", bufs=2)
                nc.vector.tensor_scalar(
                    out=wg[:, :W], in0=eband[:, :W], scalar1=rinv[:, 0:1],
                    scalar2=None, op0=AT.mult)

                # wgT via PE transpose
                wgT = {}
                for kk in need_b[g]:
                    trp = ps.tile([128, 128], BF, name="tr", tag="tr", bufs=3)
                    nc.tensor.transpose(
                        trp[:], wg[:, kk * 128:(kk + 1) * 128], ident_t[:])
                    t = gp.tile([128, 128], BF, name=f"wgT{kk}", tag=f"wgT{kk}",
                                bufs=2)
                    nc.vector.tensor_copy(out=t[:], in_=trp[:])
                    wgT[kk] = t

                # h1[s, hid] accumulation and relu
                steps = ([(ohS[kk], Psb[0][p0 + kk]) for kk in need_s[g]]
                         + [(ohE[kk], Psb[1][p0 + kk]) for kk in need_e[g]]
                         + [(ohlT, WB_t)]
                         + [(wgT[kk], Psb[2][p0 + kk]) for kk in need_b[g]])
                # h1 relu into TWO per-half tiles: whole-tile dependency
                # tracking otherwise makes every transpose wait on the LAST
                # relu ACT; split tiles let hc0-3 transpose immediately while
                # the second half's ACT finishes underneath them
                h1bh = [gp.tile([128, 512], BF, name=f"h1bh{i}",
                                tag=f"h1bh{i}", bufs=2) for i in range(2)]
                for hi, h0 in enumerate((0, 512)):
                    hp = ps.tile([128, 512], F32, name="big", tag="big", bufs=4)
                    for i, (lt, rt) in enumerate(steps):
                        nc.tensor.matmul(hp[:], lt[:], rt[:, h0:h0 + 512],
                                         start=(i == 0),
                                         stop=(i == len(steps) - 1))
                    nc.scalar.activation(h1bh[hi][:], hp[:], AF.Relu)
                for hc in range(8):
                    trp = ps.tile([128, 128], BF, name="tr", tag="tr", bufs=3)
                    nc.tensor.transpose(
                        trp[:], h1bh[hc // 4][:, (hc % 4) * 128:
                                              (hc % 4 + 1) * 128], ident_t[:])
                    dst = h1bT[hc][:, gcol:gcol + 128]
                    if hc % 2 == 0:
                        nc.scalar.copy(dst, trp[:])
                    else:
                        nc.vector.tensor_copy(out=dst, in_=trp[:])

                # every 4 groups: span-MLP L2+L3 on the 512-col block
                if g % 4 == 3:
                    b0 = (g // 4) * 512
                    h2b = [gp.tile([128, 512], BF, name=f"h2b{k}",
                                   tag=f"h2b{k}", bufs=1) for k in range(8)]
                    for h2c in range(8):
                        pt = ps.tile([128, 512], F32, name="big", tag="big",
                                     bufs=4)
                        for k in range(8):
                            nc.tensor.matmul(
                                pt[:], w2_t[k][:, h2c * 128:(h2c + 1) * 128],
                                h1bT[k][:], start=(k == 0), stop=(k == 7))
                        nc.scalar.activation(
                            h2b[h2c][:], pt[:], AF.Relu,
                            bias=b2_t[:, h2c:h2c + 1])
                    pt1 = ps.tile([1, 512], F32, name="big1", tag="big1",
                                  bufs=1)
                    for k in range(8):
                        nc.tensor.matmul(pt1[:], w3_t[:, k:k + 1], h2b[k][:],
                                         start=(k == 0), stop=(k == 7))
                    ob = gp.tile([1, 512], F32, name="ob", tag="ob", bufs=2)
                    nc.vector.tensor_scalar(out=ob[:], in0=pt1[:],
                                            scalar1=float(b3val), scalar2=None,
                                            op0=AT.add)
                    dma(out=scores_p[:, b0:b0 + 512], in_=ob[:])

    _split_waits(nc)
    return nc


def _split_waits(nc, max_waits=1):
    """This walrus build rejects instructions carrying >max_waits sem waits
    ("Too many sync wait commands"). Hoist excess waits onto same-engine
    NoOps placed immediately before the instruction — identical semantics
    (engine queues are in-order)."""
    ctr = [0]
    for f in nc.m.functions:
        for blk in f.blocks:
            out = []
            for ins in blk.instructions:
                si = getattr(ins, "sync_info", None)
                if si is not None and si.on_wait and len(si.on_wait) > max_waits:
                    waits = list(si.on_wait)
                    for w in waits[:-max_waits]:
                        ctr[0] += 1
                        nop = mybir.InstNoOp(
                            name=f"I-wsplit-{ctr[0]}", ins=[], outs=[],
                            sync_info=mybir.SyncInfo(on_wait=[w], on_update=[]),
                        )
                        nop.engine = ins.engine
                        out.append(nop)
                    ins.sync_info = mybir.SyncInfo(
                        on_wait=waits[-max_waits:],
                        on_update=list(si.on_update or []),
                    )
                out.append(ins)
            blk.instructions[:] = out
    return ctr[0]


_CACHE = {}
LAST_EXEC_NS = None
TRACE = False


def _install_ntff_shim():
    try:
        import antenv.axon_hooks  # noqa: F401
        return
    except ImportError:
        pass
    try:
        from trn_agent_boot.trn_boot import _ntff_profile_via_ctypes
        hook = _ntff_profile_via_ctypes("/opt/axon/libaxon_pjrt.so")
    except Exception:
        hook = None
    m1 = types.ModuleType("antenv")
    m2 = types.ModuleType("antenv.axon_hooks")
    m2.get_axon_ntff_profile_hook = lambda: hook
    m2.set_axon_ntff_profile_hook = lambda h: None
    m1.axon_hooks = m2
    sys.modules.setdefault("antenv", m1)
    sys.modules["antenv.axon_hooks"] = m2


def _prepare(inputs):
    inp = {k: np.asarray(v) for k, v in inputs.items()}
    ss = inp["span_starts"].astype(np.int64)
    sl = inp["span_lengths"].astype(np.int64)
    plan = _plan(ss, sl)
    T_cap, K_WIN = plan["T_cap"], plan["K_WIN"]
    KC = K_WIN // 128
    b3val = float(np.asarray(inp["score_b3"]).reshape(-1)[0])

    key = (T_cap, K_WIN, tuple(plan["bases"]), tuple(plan["kcs"]),
           plan["need_s"], plan["need_e"], plan["need_b"], b3val, FP8_ATTN)
    if key not in _CACHE:
        _CACHE[key] = _build(plan, b3val)
    nc = _CACHE[key]

    def bfc(x):
        return np.ascontiguousarray(np.asarray(x, dtype=np.float32)).astype(bf16)

    def f8c(x):
        return np.ascontiguousarray(np.asarray(x, dtype=np.float32)).astype(f8np)

    sw1 = inp["score_w1"].astype(np.float32)
    shared = {
        "aw3m": bfc(inp["attn_w3"].reshape(8, 128).T),
        "ab1m": np.ascontiguousarray(
            inp["attn_b1"].astype(np.float32).reshape(8, 128).T),
        "ab2m": np.ascontiguousarray(
            inp["attn_b2"].astype(np.float32).reshape(8, 128).T),
        "w1a": bfc(sw1[0:1024]),
        "w1b": bfc(sw1[1024:2048]),
        "w1c": bfc(sw1[2048:3072]),
        "w1d": bfc(sw1[3072:3092]),
        "wtT": bfc(inp["width_table"].T),
        "b1r": bfc(inp["score_b1"].reshape(1, HID)),
        "w2": bfc(inp["score_w2"]),
        "b2m": np.ascontiguousarray(
            inp["score_b2"].astype(np.float32).reshape(8, 128).T),
        "w3m": bfc(inp["score_w3"].reshape(8, 128).T),
        "iotaKf": np.arange(K_WIN, dtype=np.float32).reshape(1, -1),
        "iotaC": np.ascontiguousarray(
            (np.arange(128, dtype=np.float32)[:, None]
             + 128.0 * np.arange(KC, dtype=np.float32)[None, :])),
        "ident": np.eye(128, dtype=np.float32).astype(bf16),
    }
    if FP8_ATTN:
        shared["aw1"] = f8c(inp["attn_w1"])
        shared["aw2"] = f8c(inp["attn_w2"])
    else:
        shared["aw1"] = bfc(inp["attn_w1"])
        shared["aw2"] = bfc(inp["attn_w2"])

    states = inp["states"].astype(np.float32)
    embeds = inp["embeds"].astype(np.float32)
    in_maps = []
    for c in range(N_CORES):
        cb = int(plan["core_base"][c])
        stl = np.zeros((T_cap, D), np.float32)
        eml = np.zeros((T_cap, D), np.float32)
        hi = min(T, cb + T_cap)
        stl[: hi - cb] = states[cb:hi]
        eml[: hi - cb] = embeds[cb:hi]
        m = dict(shared)
        sT = np.ascontiguousarray(stl.T)
        m["statesT"] = sT.astype(bf16)
        m["embedsT"] = np.ascontiguousarray(eml.T).astype(bf16)
        if FP8_ATTN:
            m["sT8"] = sT.astype(f8np)
        d = plan["d"][c].astype(np.float32)
        dl = plan["dl"][c].astype(np.float32)
        ln = plan["ln"][c].astype(np.float32)
        m["dmat"] = np.ascontiguousarray(d.reshape(G, 128).T)
        m["dlmat"] = np.ascontiguousarray(dl.reshape(G, 128).T)
        dde = np.stack([d.reshape(G, 128), dl.reshape(G, 128)], axis=1)
        m["ddeflat"] = np.ascontiguousarray(dde.reshape(1, 2 * C))
        m["lenflat"] = ln.reshape(1, C)
        in_maps.append(m)

    return nc, in_maps, plan


def kernel(**inputs):
    global LAST_EXEC_NS
    from concourse.bass_utils import run_bass_kernel_spmd

    nc, in_maps, plan = _prepare(inputs)
    _install_ntff_shim()
    res = run_bass_kernel_spmd(nc, in_maps, list(range(N_CORES)), trace=TRACE)
    LAST_EXEC_NS = res.exec_time_ns

    out = np.empty(NSPAN, np.float32)
    for c in range(N_CORES):
        out[plan["order"][c * C:(c + 1) * C]] = np.asarray(
            res.results[c]["scores"]).reshape(-1)
    return out.reshape(NSPAN, 1)


# revision 45
# speedup vs baseline: 1.1011x; 1.0297x over previous
"""Trainium2 Bass kernel for nn_MentionScore (v2).

Strategy: sort spans by start, shard 2048 consecutive sorted spans per core.
Each core touches a ~1.2k-token window of states/embeds. Layer-1 of the span
MLP is folded into per-token projections:
  h1[s] = relu(P1[start_s] + P2[end_s] + sum_t wg[s,t] P3[t] + WB[len_s])
with P1=states@W1a, P2=states@W1b, P3=embeds@W1c and WB=width_table@W1d+b1.

v2 changes vs baseline:
- P1/P2/P3 stay resident in SBUF (group windows 128-aligned); no DRAM
  round-trip for the projections.
- Span-group gathers run with the one-hot as the stationary matmul operand
  (f=512 moving), cutting LDWEIGHTS pressure ~4x; h1 is transposed back for
  layer 2 with PE transposes.
- Softmax built from the band identity exp(sa[s,l]) = exp(attns[start_s+l]):
  exp is taken once per token in the token pipeline; per group the weight
  matrix is band(d<=tau<=d+len)*exp(attns)*rinv via 3 fused DVE ops.
- Attention MLP (L1+L2) runs in fp8 e4m3 DoubleRow (2x tensor throughput);
  validated to add <1e-3 to final error.
- relu/bias epilogues on the Scalar engine; psum copies split Scalar/Vector.
"""

import sys
import types

import numpy as np
import ml_dtypes

import concourse.bass as bass
import concourse.mybir as mybir
from concourse.ap import AP
from concourse.tile import TileContext
from concourse.vector_clock import ScopedClock

BF = mybir.dt.bfloat16
F32 = mybir.dt.float32
F8 = mybir.dt.float8e4
AT = mybir.AluOpType
AF = mybir.ActivationFunctionType
DR = mybir.MatmulPerfMode.DoubleRow
bf16 = ml_dtypes.bfloat16
f8np = mybir.dt.np(F8)

N_CORES = 8
T, NSPAN, D, HID, LMAX, WD = 8192, 16384, 1024, 1024, 10, 20
C = NSPAN // N_CORES          # spans per core
G = C // 128                  # 128-span groups per core

FP8_ATTN = True


class PatchedTileContext(TileContext):
    """Workaround: walrus rejects the tail Drain when it carries >1 sem wait
    ("Too many sync wait commands"). Put each wait on its own NoOp instead."""

    def _drain_and_barrier(self, tick_clock, wait_clock):
        nc = self.nc
        drain_inst = nc.sync.drain()
        wait_clock.add_sem_waits(
            drain_inst.ins, ScopedClock({None: tick_clock.global_clock})
        )
        si = drain_inst.ins.sync_info
        if si is not None and si.on_wait is not None and len(si.on_wait) > 1:
            waits = list(si.on_wait)
            drain_inst.ins.sync_info = mybir.SyncInfo(
                on_wait=[waits[0]], on_update=list(si.on_update or [])
            )
            for w in waits[1:]:
                nop = nc.sync.nop()
                nop.ins.sync_info = mybir.SyncInfo(on_wait=[w], on_update=[])

        nc.all_engine_barrier()
        assert self.sems is not None
        popped = nc._tile_sem_poison_stack.pop()
        assert popped is self._sem_poison
        nc.clear_and_free_semaphores(list(self.sems.allocated().values()))
        nc.all_engine_barrier()


def _ceil128(x):
    return int(-(-int(x) // 128) * 128)


def _plan(span_starts, span_lengths):
    """Host-side sharding plan. Returns per-core data + static layout consts."""
    order = np.argsort(span_starts, kind="stable").astype(np.int64)
    ss = span_starts[order].reshape(N_CORES, C).astype(np.int64)
    sl = span_lengths[order].reshape(N_CORES, C).astype(np.int64)
    core_base = ss[:, 0].copy()
    sloc = ss - core_base[:, None]
    eloc = sloc + sl

    T_cap = _ceil128(int(eloc.max()) + 1)
    # 128-aligned, shared-across-cores group window bases
    mn = sloc[:, ::128].min(axis=0)                             # [G]
    mx = eloc.reshape(N_CORES, G, 128).max(axis=2).max(axis=0)  # [G]
    bases = (mn // 128) * 128
    kcs = -(-(mx - bases + 1) // 128)
    K_WIN = int(kcs.max()) * 128
    T_pad = max(T_cap, int((bases + kcs * 128).max()))
    d = sloc - np.repeat(bases, 128)[None, :]
    dl = d + sl
    assert d.min() >= 0 and (dl.reshape(N_CORES, G, 128).max(axis=2)
                             <= kcs[None, :] * 128 - 1).all(), "window overflow"

    # static pruning lists (shared program => OR over cores)
    need_s, need_e, need_b = [], [], []
    for g in range(G):
        dg = d[:, g * 128:(g + 1) * 128]
        dlg = dl[:, g * 128:(g + 1) * 128]
        ns, ne, nb = [], [], []
        for kk in range(int(kcs[g])):
            lo, hi = kk * 128, kk * 128 + 127
            if ((dg >= lo) & (dg <= hi)).any():
                ns.append(kk)
            if ((dlg >= lo) & (dlg <= hi)).any():
                ne.append(kk)
            if ((dg <= hi) & (dlg >= lo)).any():
                nb.append(kk)
        need_s.append(tuple(ns))
        need_e.append(tuple(ne))
        need_b.append(tuple(nb))

    return {
        "order": order,
        "core_base": core_base,
        "d": d.astype(np.float64),
        "dl": dl.astype(np.float64),
        "ln": sl.astype(np.float64),
        "T_cap": T_cap,
        "T_pad": int(T_pad),
        "K_WIN": int(K_WIN),
        "bases": [int(b) for b in bases],
        "kcs": [int(k) for k in kcs],
        "need_s": tuple(need_s),
        "need_e": tuple(need_e),
        "need_b": tuple(need_b),
    }


def _build(plan, b3val):
    """Build the single SPMD Bass program (static; shared by all 8 cores)."""
    T_cap = plan["T_cap"]
    K_WIN = plan["K_WIN"]
    bases = plan["bases"]
    kcs = plan["kcs"]
    need_s, need_e, need_b = plan["need_s"], plan["need_e"], plan["need_b"]
    TC = T_cap // 128
    KC = K_WIN // 128
    nc = bass.Bass()

    def par(name, shape, dt):
        return nc.declare_dram_parameter(name, list(shape), dt, isOutput=False)

    statesT_p = par("statesT", [D, T_cap], BF)
    embedsT_p = par("embedsT", [D, T_cap], BF)
    if FP8_ATTN:
        sT8_p = par("sT8", [D, T_cap], F8)
        aw1_p = par("aw1", [D, HID], F8)
        aw2_p = par("aw2", [HID, HID], F8)
    else:
        aw1_p = par("aw1", [D, HID], BF)
        aw2_p = par("aw2", [HID, HID], BF)
    aw3_p = par("aw3m", [128, 8], BF)
    ab1_p = par("ab1m", [128, 8], F32)
    ab2_p = par("ab2m", [128, 8], F32)
    w1a_p = par("w1a", [D, HID], BF)
    w1b_p = par("w1b", [D, HID], BF)
    w1c_p = par("w1c", [D, HID], BF)
    w1d_p = par("w1d", [WD, HID], BF)
    wtT_p = par("wtT", [WD, LMAX], BF)
    b1r_p = par("b1r", [1, HID], BF)
    w2_p = par("w2", [HID, HID], BF)
    b2_p = par("b2m", [128, 8], F32)
    w3_p = par("w3m", [128, 8], BF)
    dde_p = par("ddeflat", [1, 2 * C], F32)
    dmat_p = par("dmat", [128, G], F32)
    dlmat_p = par("dlmat", [128, G], F32)
    lenflat_p = par("lenflat", [1, C], F32)
    iotaK_p = par("iotaKf", [1, K_WIN], F32)
    iotaC_p = par("iotaC", [128, KC], F32)
    ident_p = par("ident", [128, 128], BF)
    scores_p = nc.declare_dram_parameter("scores", [1, C], F32, isOutput=True)

    with PatchedTileContext(nc) as tc:
        with (
            tc.tile_pool(name="pp", bufs=1) as pp,
            tc.tile_pool(name="wst", bufs=1) as wst,
            tc.tile_pool(name="gp", bufs=2) as gp,
            tc.tile_pool(name="ps", bufs=1, space="PSUM") as ps,
            tc.tile_pool(name="dp", bufs=1, space="DRAM") as dp,
        ):
            dma = nc.sync.dma_start
            sdma = dma  # scalar-queue triggers stall ACTs; keep all on sync
            nblocks = [(n0, min(512, T_cap - n0)) for n0 in range(0, T_cap, 512)]

            # ---------- startup-critical DMAs first ----------
            # sT8 on sync, aw1 on scalar: both HWDGE queue groups in parallel
            def load_s8(n0, nw, eng):
                s8 = []
                for kc in range(4):
                    t = wst.tile([128, 2, 512], F8, name=f"sT8w{kc}",
                                 tag=f"sT8w{kc}", bufs=1)
                    src = AP(tensor=sT8_p[:].tensor,
                             offset=kc * 256 * T_cap + n0,
                             ap=[[T_cap, 128], [128 * T_cap, 2], [1, nw]])
                    eng(out=t[:, :, :nw], in_=src)
                    s8.append(t)
                return s8

            blk_tiles = {}
            if FP8_ATTN:
                blk_tiles["s8"] = load_s8(nblocks[0][0], nblocks[0][1], dma)
                aw1_t = []
                for kc in range(4):
                    t = pp.tile([128, 2, HID], F8, name=f"aw1_{kc}",
                                tag=f"aw1_{kc}")
                    src = AP(tensor=aw1_p[:].tensor, offset=kc * 256 * HID,
                             ap=[[HID, 128], [128 * HID, 2], [1, HID]])
                    sdma(out=t[:], in_=src)
                    aw1_t.append(t)
            else:
                aw1_t = []
                for k in range(8):
                    t = pp.tile([128, HID], BF, name=f"aw1_{k}", tag=f"aw1_{k}")
                    (dma if k % 2 else sdma)(
                        out=t[:], in_=aw1_p[k * 128:(k + 1) * 128, :])
                    aw1_t.append(t)

            # small biases needed by the first activations
            ab1_t = pp.tile([128, 8], F32, name="ab1", tag="ab1")
            dma(out=ab1_t[:], in_=ab1_p[:])
            ab2_t = pp.tile([128, 8], F32, name="ab2", tag="ab2")
            dma(out=ab2_t[:], in_=ab2_p[:])
            aw3_t = pp.tile([128, 8], BF, name="aw3", tag="aw3")
            dma(out=aw3_t[:], in_=aw3_p[:])

            # attn L2 weights (needed ~15us in)
            if FP8_ATTN:
                aw2_t = []
                for kc in range(4):
                    t = pp.tile([128, 2, HID], F8, name=f"aw2_{kc}",
                                tag=f"aw2_{kc}")
                    src = AP(tensor=aw2_p[:].tensor, offset=kc * 256 * HID,
                             ap=[[HID, 128], [128 * HID, 2], [1, HID]])
                    (dma if kc % 2 else sdma)(out=t[:], in_=src)
                    aw2_t.append(t)
            else:
                aw2_t = []
                for k in range(8):
                    t = pp.tile([128, HID], BF, name=f"aw2_{k}", tag=f"aw2_{k}")
                    (dma if k % 2 else sdma)(
                        out=t[:], in_=aw2_p[k * 128:(k + 1) * 128, :])
                    aw2_t.append(t)

            def wload(param, tag_prefix):
                tiles = []
                for k in range(8):
                    t = pp.tile([128, HID], BF, name=f"{tag_prefix}{k}",
                                tag=f"{tag_prefix}{k}")
                    (dma if k % 2 else sdma)(
                        out=t[:], in_=param[k * 128:(k + 1) * 128, :])
                    tiles.append(t)
                return tiles

            # P weights in first-use order (P loop below is pi-major)
            w1a_t = wload(w1a_p, "wWA")

            def load_se(n0, nw, which):
                tiles = []
                for k in range(8):
                    ts_ = wst.tile([128, 512], BF, name=f"{which}{k}",
                                   tag=f"{which}{k}", bufs=1)
                    p_ = statesT_p if which == "sTw" else embedsT_p
                    (dma if k % 2 else sdma)(
                        out=ts_[:, :nw],
                        in_=p_[k * 128:(k + 1) * 128, n0:n0 + nw])
                    tiles.append(ts_)
                return tiles

            blk_tiles["sTw"] = load_se(nblocks[0][0], nblocks[0][1], "sTw")
            w1b_t = wload(w1b_p, "wWB")
            blk_tiles["eTw"] = load_se(nblocks[0][0], nblocks[0][1], "eTw")
            w1c_t = wload(w1c_p, "wWC")

            def load_block(n0, nw):
                tiles = {}
                if FP8_ATTN:
                    tiles["s8"] = load_s8(n0, nw, dma)
                tiles["sTw"] = load_se(n0, nw, "sTw")
                tiles["eTw"] = load_se(n0, nw, "eTw")
                return tiles

            # ---------- constants / scalars ----------
            iotaK_t = pp.tile([128, K_WIN], F32, name="iotaK", tag="iotaK")
            dma(out=iotaK_t[:], in_=iotaK_p[:].partition_broadcast(128))
            iotaC_t = pp.tile([128, KC], F32, name="iotaC", tag="iotaC")
            dma(out=iotaC_t[:], in_=iotaC_p[:])
            ident_t = pp.tile([128, 128], BF, name="ident", tag="ident")
            dma(out=ident_t[:], in_=ident_p[:])
            ones16_t = pp.tile([1, 16], BF, name="ones16", tag="ones16")
            nc.vector.memset(ones16_t[:], 1.0)
            dmat_t = pp.tile([128, G], F32, name="dmat", tag="dmat")
            dma(out=dmat_t[:], in_=dmat_p[:])
            dlmat_t = pp.tile([128, G], F32, name="dlmat", tag="dlmat")
            dma(out=dlmat_t[:], in_=dlmat_p[:])
            b2_t = pp.tile([128, 8], F32, name="b2", tag="b2")
            dma(out=b2_t[:], in_=b2_p[:])
            w3_t = pp.tile([128, 8], BF, name="w3", tag="w3")
            dma(out=w3_t[:], in_=w3_p[:])
            b1r_t = pp.tile([1, HID], BF, name="b1r", tag="b1r")
            dma(out=b1r_t[:], in_=b1r_p[:])
            w1d_t = pp.tile([WD, HID], BF, name="w1d", tag="w1d")
            dma(out=w1d_t[:], in_=w1d_p[:])
            wtT_t = pp.tile([WD, 16], BF, name="wtT", tag="wtT")
            nc.vector.memset(wtT_t[:], 0.0)
            dma(out=wtT_t[:, :LMAX], in_=wtT_p[:])

            eat_dram = dp.tile([1, T_cap + K_WIN], BF, name="eat_dram",
                               tag="eat_dram")

            # ---------- P projection SBUF residents ----------
            Psb = []
            for pi in range(3):
                Psb.append([pp.tile([128, HID], BF, name=f"P{pi}_{jt}",
                                    tag=f"P{pi}_{jt}") for jt in range(TC)])

            eat_t = pp.tile([1, T_cap], BF, name="eat", tag="eat")

            # ---------- token pipeline ----------
            for bi, (n0, nw) in enumerate(nblocks):
                cur = blk_tiles
                sTw, eTw = cur["sTw"], cur["eTw"]
                # attn L1 (kc-outer so block-0 compute starts on the first
                # weight k-chunk instead of waiting for the full matrix)
                if FP8_ATTN:
                    h1a8 = [wst.tile([128, 2, 512], F8, name=f"h1a8{k}",
                                     tag=f"h1a8{k}", bufs=1) for k in range(4)]
                    for hq in (0, 4):
                        pts = [ps.tile([128, 512], F32, name="big", tag="big",
                                       bufs=4) for _ in range(4)]
                        for kc in range(4):
                            for hi_ in range(4):
                                hc = hq + hi_
                                nc.tensor.matmul(
                                    pts[hi_][:, :nw],
                                    aw1_t[kc][:, :, hc * 128:(hc + 1) * 128],
                                    cur["s8"][kc][:, :, :nw],
                                    start=(kc == 0), stop=(kc == 3),
                                    perf_mode=DR, skip_group_check=True)
                        for hi_ in range(4):
                            hc = hq + hi_
                            nc.scalar.activation(
                                h1a8[hc // 2][:, hc % 2, :nw], pts[hi_][:, :nw],
                                AF.Relu, bias=ab1_t[:, hc:hc + 1])
                else:
                    h1a = [wst.tile([128, 512], BF, name=f"h1a{k}",
                                    tag=f"h1a{k}", bufs=1) for k in range(8)]
                    for hc in range(8):
                        pt = ps.tile([128, 512], F32, name="big", tag="big",
                                     bufs=4)
                        for k in range(8):
                            nc.tensor.matmul(
                                pt[:, :nw],
                                aw1_t[k][:, hc * 128:(hc + 1) * 128],
                                sTw[k][:, :nw], start=(k == 0), stop=(k == 7))
                        nc.scalar.activation(
                            h1a[hc][:, :nw], pt[:, :nw], AF.Relu,
                            bias=ab1_t[:, hc:hc + 1])
                # prefetch next block inputs
                if bi + 1 < len(nblocks):
                    blk_tiles = load_block(*nblocks[bi + 1])
                # attn L2
                h2a = [wst.tile([128, 512], BF, name=f"h2a{k}", tag=f"h2a{k}",
                                bufs=1) for k in range(8)]
                if FP8_ATTN:
                    for hq in (0, 4):
                        pts = [ps.tile([128, 512], F32, name="big", tag="big",
                                       bufs=4) for _ in range(4)]
                        for kc in range(4):
                            for hi_ in range(4):
                                hc = hq + hi_
                                nc.tensor.matmul(
                                    pts[hi_][:, :nw],
                                    aw2_t[kc][:, :, hc * 128:(hc + 1) * 128],
                                    h1a8[kc][:, :, :nw],
                                    start=(kc == 0), stop=(kc == 3),
                                    perf_mode=DR, skip_group_check=True)
                        for hi_ in range(4):
                            hc = hq + hi_
                            nc.scalar.activation(
                                h2a[hc][:, :nw], pts[hi_][:, :nw], AF.Relu,
                                bias=ab2_t[:, hc:hc + 1])
                else:
                    for hc in range(8):
                        pt = ps.tile([128, 512], F32, name="big", tag="big",
                                     bufs=4)
                        for k in range(8):
                            nc.tensor.matmul(
                                pt[:, :nw],
                                aw2_t[k][:, hc * 128:(hc + 1) * 128],
                                h1a[k][:, :nw], start=(k == 0), stop=(k == 7))
                        nc.scalar.activation(
                            h2a[hc][:, :nw], pt[:, :nw], AF.Relu,
                            bias=ab2_t[:, hc:hc + 1])
                # attn w3 -> exp -> eat
                pt1 = ps.tile([128, 512], F32, name="big", tag="big",
                              bufs=4)
                for k in range(8):
                    nc.tensor.matmul(pt1[:1, :nw], aw3_t[:, k:k + 1],
                                     h2a[k][:, :nw],
                                     start=(k == 0), stop=(k == 7))
                nc.scalar.activation(eat_t[0:1, n0:n0 + nw], pt1[:1, :nw],
                                     AF.Exp)
                dma(out=eat_dram[0:1, n0:n0 + nw], in_=eat_t[0:1, n0:n0 + nw])
                # P projections into SBUF residents (pi-major: matches the
                # w1a -> w1b -> w1c weight-arrival order at startup)
                for pi, (wt_, srcs) in enumerate(
                        ((w1a_t, sTw), (w1b_t, sTw), (w1c_t, eTw))):
                    for j in range(nw // 128):
                        js = slice(j * 128, (j + 1) * 128)
                        jt = (n0 + j * 128) // 128
                        for h0 in (0, 512):
                            pt = ps.tile([128, 512], F32, name="big",
                                         tag="big", bufs=4)
                            for k in range(8):
                                nc.tensor.matmul(
                                    pt[:], srcs[k][:, js],
                                    wt_[k][:, h0:h0 + 512],
                                    start=(k == 0), stop=(k == 7))
                            dst = Psb[pi][jt][:, h0:h0 + 512]
                            if pi == 2:
                                nc.scalar.copy(dst, pt[:])
                            else:
                                nc.vector.tensor_copy(out=dst, in_=pt[:])

            # zero-pad eat beyond T_cap (bands never reach there, but NaN-safe)
            zpad_t = pp.tile([1, K_WIN], BF, name="zpad", tag="zpad")
            nc.vector.memset(zpad_t[:], 0.0)
            dma(out=eat_dram[0:1, T_cap:], in_=zpad_t[0:1, :])

            # ---------- WB = width_table @ W1d + b1 -> [16, HID] ----------
            WB_t = pp.tile([16, HID], BF, name="WB", tag="WB")
            for h0 in range(0, HID, 512):
                pt = ps.tile([128, 512], F32, name="big", tag="big", bufs=4)
                nc.tensor.matmul(pt[:16, :], wtT_t[:], w1d_t[:, h0:h0 + 512],
                                 start=True, stop=False)
                nc.tensor.matmul(pt[:16, :], ones16_t[:], b1r_t[:, h0:h0 + 512],
                                 start=False, stop=True)
                nc.vector.tensor_copy(out=WB_t[:, h0:h0 + 512], in_=pt[:16, :])

            # span-MLP L2 weights reuse the w1a slots
            w2_t = wload(w2_p, "wWA")

            # ---------- span groups ----------
            h1bT = None
            for g in range(G):
                W = kcs[g] * 128
                p0 = bases[g] // 128
                if g % 4 == 0:
                    h1bT = [gp.tile([128, 512], BF, name=f"h1bT{k}",
                                    tag=f"h1bT{k}", bufs=2) for k in range(8)]
                gcol = (g % 4) * 128

                dde = gp.tile([128, 256], F32, name="dde", tag="dde", bufs=2)
                dma(out=dde[:],
                    in_=dde_p[:, g * 256:(g + 1) * 256].partition_broadcast(128))
                eat_rep = gp.tile([128, K_WIN], BF, name="eat_rep",
                                  tag="eat_rep", bufs=2)
                dma(out=eat_rep[:, :W],
                    in_=eat_dram[0:1, bases[g]:bases[g] + W]
                    .partition_broadcast(128))
                len_rep = gp.tile([16, 128], F32, name="len_rep", tag="len_rep",
                                  bufs=2)
                dma(out=len_rep[:],
                    in_=lenflat_p[:, g * 128:(g + 1) * 128]
                    .partition_broadcast(16))

                # one-hot tiles [tau, s]
                ohS = {}
                for kk in need_s[g]:
                    t = gp.tile([128, 128], BF, name=f"ohS{kk}", tag=f"ohS{kk}",
                                bufs=2)
                    nc.vector.tensor_scalar(
                        out=t[:], in0=dde[:, :128],
                        scalar1=iotaC_t[:, kk:kk + 1], scalar2=None,
                        op0=AT.is_equal)
                    ohS[kk] = t
                ohE = {}
                for kk in need_e[g]:
                    t = gp.tile([128, 128], BF, name=f"ohE{kk}", tag=f"ohE{kk}",
                                bufs=2)
                    nc.vector.tensor_scalar(
                        out=t[:], in0=dde[:, 128:256],
                        scalar1=iotaC_t[:, kk:kk + 1], scalar2=None,
                        op0=AT.is_equal)
                    ohE[kk] = t
                ohlT = gp.tile([16, 128], BF, name="ohlT", tag="ohlT", bufs=2)
                nc.vector.tensor_scalar(
                    out=ohlT[:], in0=len_rep[:], scalar1=iotaC_t[:16, 0:1],
                    scalar2=None, op0=AT.is_equal)

                # wg [s, tau] = band * exp(attns) * rinv  (3 fused DVE ops)
                t2e = gp.tile([128, K_WIN], BF, name="t2e", tag="t2e", bufs=2)
                nc.vector.scalar_tensor_tensor(
                    out=t2e[:, :W], in0=iotaK_t[:, :W],
                    scalar=dlmat_t[:, g:g + 1], in1=eat_rep[:, :W],
                    op0=AT.is_le, op1=AT.mult)
                eband = gp.tile([128, K_WIN], BF, name="eband", tag="eband",
                                bufs=2)
                ssum = gp.tile([128, 1], F32, name="ssum", tag="ssum", bufs=2)
                nc.vector.scalar_tensor_tensor(
                    out=eband[:, :W], in0=iotaK_t[:, :W],
                    scalar=dmat_t[:, g:g + 1], in1=t2e[:, :W],
                    op0=AT.is_ge, op1=AT.mult, accum_out=ssum[:, 0:1])
                rinv = gp.tile([128, 1], F32, name="rinv", tag="rinv", bufs=2)
                nc.vector.reciprocal(rinv[:], ssum[:])
                wg = gp.tile([128, K_WIN], BF, name="wg", tag="wg", bufs=2)
                nc.vector.tensor_scalar(
                    out=wg[:, :W], in0=eband[:, :W], scalar1=rinv[:, 0:1],
                    scalar2=None, op0=AT.mult)

                # wgT via PE transpose
                wgT = {}
                for kk in need_b[g]:
                    trp = ps.tile([128, 128], BF, name="tr", tag="tr", bufs=4)
                    nc.tensor.transpose(
                        trp[:], wg[:, kk * 128:(kk + 1) * 128], ident_t[:])
                    t = gp.tile([128, 128], BF, name=f"wgT{kk}", tag=f"wgT{kk}",
                                bufs=2)
                    nc.vector.tensor_copy(out=t[:], in_=trp[:])
                    wgT[kk] = t

                # h1[s, hid] accumulation and relu
                steps = ([(ohS[kk], Psb[0][p0 + kk]) for kk in need_s[g]]
                         + [(ohE[kk], Psb[1][p0 + kk]) for kk in need_e[g]]
                         + [(ohlT, WB_t)]
                         + [(wgT[kk], Psb[2][p0 + kk]) for kk in need_b[g]])
                # h1 relu into TWO per-half tiles: whole-tile dependency
                # tracking otherwise makes every transpose wait on the LAST
                # relu ACT; split tiles let hc0-3 transpose immediately while
                # the second half's ACT finishes underneath them
                h1bh = [gp.tile([128, 512], BF, name=f"h1bh{i}",
                                tag=f"h1bh{i}", bufs=2) for i in range(2)]
                for hi, h0 in enumerate((0, 512)):
                    hp = ps.tile([128, 512], F32, name="big", tag="big", bufs=4)
                    for i, (lt, rt) in enumerate(steps):
                        nc.tensor.matmul(hp[:], lt[:], rt[:, h0:h0 + 512],
                                         start=(i == 0),
                                         stop=(i == len(steps) - 1))
                    nc.scalar.activation(h1bh[hi][:], hp[:], AF.Relu)
                for hc in range(8):
                    trp = ps.tile([128, 128], BF, name="tr", tag="tr", bufs=4)
                    nc.tensor.transpose(
                        trp[:], h1bh[hc // 4][:, (hc % 4) * 128:
                                              (hc % 4 + 1) * 128], ident_t[:])
                    dst = h1bT[hc][:, gcol:gcol + 128]
                    if hc % 2 == 0:
                        nc.scalar.copy(dst, trp[:])
                    else:
                        nc.vector.tensor_copy(out=dst, in_=trp[:])

                # every 4 groups: span-MLP L2+L3 on the 512-col block
                if g % 4 == 3:
                    b0 = (g // 4) * 512
                    h2b = [gp.tile([128, 512], BF, name=f"h2b{k}",
                                   tag=f"h2b{k}", bufs=1) for k in range(8)]
                    for h2c in range(8):
                        pt = ps.tile([128, 512], F32, name="big", tag="big",
                                     bufs=4)
                        for k in range(8):
                            nc.tensor.matmul(
                                pt[:], w2_t[k][:, h2c * 128:(h2c + 1) * 128],
                                h1bT[k][:], start=(k == 0), stop=(k == 7))
                        nc.scalar.activation(
                            h2b[h2c][:], pt[:], AF.Relu,
                            bias=b2_t[:, h2c:h2c + 1])
                    pt1 = ps.tile([128, 512], F32, name="big", tag="big",
                                  bufs=4)
                    for k in range(8):
                        nc.tensor.matmul(pt1[:1, :], w3_t[:, k:k + 1],
                                         h2b[k][:],
                                         start=(k == 0), stop=(k == 7))
                    ob = gp.tile([1, 512], F32, name="ob", tag="ob", bufs=2)
                    nc.vector.tensor_scalar(out=ob[:], in0=pt1[:1, :],
                                            scalar1=float(b3val), scalar2=None,
                                            op0=AT.add)
                    dma(out=scores_p[:, b0:b0 + 512], in_=ob[:])

    _split_waits(nc)
    return nc


def _split_waits(nc, max_waits=1):
    """This walrus build rejects instructions carrying >max_waits sem waits
    ("Too many sync wait commands"). Hoist excess waits onto same-engine
    NoOps placed immediately before the instruction — identical semantics
    (engine queues are in-order)."""
    ctr = [0]
    for f in nc.m.functions:
        for blk in f.blocks:
            out = []
            for ins in blk.instructions:
                si = getattr(ins, "sync_info", None)
                if si is not None and si.on_wait and len(si.on_wait) > max_waits:
                    waits = list(si.on_wait)
                    for w in waits[:-max_waits]:
                        ctr[0] += 1
                        nop = mybir.InstNoOp(
                            name=f"I-wsplit-{ctr[0]}", ins=[], outs=[],
                            sync_info=mybir.SyncInfo(on_wait=[w], on_update=[]),
                        )
                        nop.engine = ins.engine
                        out.append(nop)
                    ins.sync_info = mybir.SyncInfo(
                        on_wait=waits[-max_waits:],
                        on_update=list(si.on_update or []),
                    )
                out.append(ins)
            blk.instructions[:] = out
    return ctr[0]


_CACHE = {}
LAST_EXEC_NS = None
TRACE = False


def _install_ntff_shim():
    try:
        import antenv.axon_hooks  # noqa: F401
        return
    except ImportError:
        pass
    try:
        from trn_agent_boot.trn_boot import _ntff_profile_via_ctypes
        hook = _ntff_profile_via_ctypes("/opt/axon/libaxon_pjrt.so")
    except Exception:
        hook = None
    m1 = types.ModuleType("antenv")
    m2 = types.ModuleType("antenv.axon_hooks")
    m2.get_axon_ntff_profile_hook = lambda: hook
    m2.set_axon_ntff_profile_hook = lambda h: None
    m1.axon_hooks = m2
    sys.modules.setdefault("antenv", m1)
    sys.modules["antenv.axon_hooks"] = m2


def _prepare(inputs):
    inp = {k: np.asarray(v) for k, v in inputs.items()}
    ss = inp["span_starts"].astype(np.int64)
    sl = inp["span_lengths"].astype(np.int64)
    plan = _plan(ss, sl)
    T_cap, K_WIN = plan["T_cap"], plan["K_WIN"]
    KC = K_WIN // 128
    b3val = float(np.asarray(inp["score_b3"]).reshape(-1)[0])

    key = (T_cap, K_WIN, tuple(plan["bases"]), tuple(plan["kcs"]),
           plan["need_s"], plan["need_e"], plan["need_b"], b3val, FP8_ATTN)
    if key not in _CACHE:
        _CACHE[key] = _build(plan, b3val)
    nc = _CACHE[key]

    def bfc(x):
        return np.ascontiguousarray(np.asarray(x, dtype=np.float32)).astype(bf16)

    def f8c(x):
        return np.ascontiguousarray(np.asarray(x, dtype=np.float32)).astype(f8np)

    sw1 = inp["score_w1"].astype(np.float32)
    shared = {
        "aw3m": bfc(inp["attn_w3"].reshape(8, 128).T),
        "ab1m": np.ascontiguousarray(
            inp["attn_b1"].astype(np.float32).reshape(8, 128).T),
        "ab2m": np.ascontiguousarray(
            inp["attn_b2"].astype(np.float32).reshape(8, 128).T),
        "w1a": bfc(sw1[0:1024]),
        "w1b": bfc(sw1[1024:2048]),
        "w1c": bfc(sw1[2048:3072]),
        "w1d": bfc(sw1[3072:3092]),
        "wtT": bfc(inp["width_table"].T),
        "b1r": bfc(inp["score_b1"].reshape(1, HID)),
        "w2": bfc(inp["score_w2"]),
        "b2m": np.ascontiguousarray(
            inp["score_b2"].astype(np.float32).reshape(8, 128).T),
        "w3m": bfc(inp["score_w3"].reshape(8, 128).T),
        "iotaKf": np.arange(K_WIN, dtype=np.float32).reshape(1, -1),
        "iotaC": np.ascontiguousarray(
            (np.arange(128, dtype=np.float32)[:, None]
             + 128.0 * np.arange(KC, dtype=np.float32)[None, :])),
        "ident": np.eye(128, dtype=np.float32).astype(bf16),
    }
    if FP8_ATTN:
        shared["aw1"] = f8c(inp["attn_w1"])
        shared["aw2"] = f8c(inp["attn_w2"])
    else:
        shared["aw1"] = bfc(inp["attn_w1"])
        shared["aw2"] = bfc(inp["attn_w2"])

    states = inp["states"].astype(np.float32)
    embeds = inp["embeds"].astype(np.float32)
    in_maps = []
    for c in range(N_CORES):
        cb = int(plan["core_base"][c])
        stl = np.zeros((T_cap, D), np.float32)
        eml = np.zeros((T_cap, D), np.float32)
        hi = min(T, cb + T_cap)
        stl[: hi - cb] = states[cb:hi]
        eml[: hi - cb] = embeds[cb:hi]
        m = dict(shared)
        sT = np.ascontiguousarray(stl.T)
        m["statesT"] = sT.astype(bf16)
        m["embedsT"] = np.ascontiguousarray(eml.T).astype(bf16)
        if FP8_ATTN:
            m["sT8"] = sT.astype(f8np)
        d = plan["d"][c].astype(np.float32)
        dl = plan["dl"][c].astype(np.float32)
        ln = plan["ln"][c].astype(np.float32)
        m["dmat"] = np.ascontiguousarray(d.reshape(G, 128).T)
        m["dlmat"] = np.ascontiguousarray(dl.reshape(G, 128).T)
        dde = np.stack([d.reshape(G, 128), dl.reshape(G, 128)], axis=1)
        m["ddeflat"] = np.ascontiguousarray(dde.reshape(1, 2 * C))
        m["lenflat"] = ln.reshape(1, C)
        in_maps.append(m)

    return nc, in_maps, plan


def kernel(**inputs):
    global LAST_EXEC_NS
    from concourse.bass_utils import run_bass_kernel_spmd

    nc, in_maps, plan = _prepare(inputs)
    _install_ntff_shim()
    res = run_bass_kernel_spmd(nc, in_maps, list(range(N_CORES)), trace=TRACE)
    LAST_EXEC_NS = res.exec_time_ns

    out = np.empty(NSPAN, np.float32)
    for c in range(N_CORES):
        out[plan["order"][c * C:(c + 1) * C]] = np.asarray(
            res.results[c]["scores"]).reshape(-1)
    return out.reshape(NSPAN, 1)
